# revision 5
# baseline (speedup 1.0000x reference)
"""Trainium2 Bass kernel for nn_CaptioningRNN (attention LSTM, T=64 steps).

Strategy: PURE DATA-PARALLEL over N (16 samples/core, ZERO collectives).
The baseline TP design paid 2 serialized AllGathers per step (~100-190us/step
of collective latency); here every core runs its 16 samples' full recurrence
independently and only the final output is gathered on the host.

Per-core residents (SBUF, per-partition budget 192KB):
  A1  [hh, (chunk c, n, l)]      E3M4 (stores 2A)   24.5KB  - scores rhs
  A2  [lp, (lc, v=8n+blk, hh)]   E3M4 (stores 2A)   32KB    - readout rhs
  Wh  [p,  (k, 4096)]            bf16               64KB    - gates rhs
  Wa  [p,  (k, 4096)]            E4M3               32KB    - gates rhs
Mixed-dtype matmuls (bf16 lhsT x fp8 rhs) are verified exact on HW, so all
lhsT operands (h^T, exp-weights, attn^T) stay bf16.

Gates: out[16, 4096] via 4x PE column tiling (tile j = gate quarter j at
psum rows 32j..32j+16), 17 chunks each (16 z-chunks + xwb-via-identity).
LSTM pointwise runs in TRANSPOSED layout [128 h, (k, n)]: the four gate
quarters are PE-transposed per 128-chunk, which sidesteps the DVE
equal-partition-base restriction and directly yields h^T for the next
step's lhsT. Softmax is max-subtracted; 1/sum is applied post-readout via
a 0/1 replication matmul that broadcasts the per-sample reciprocal to the
128 virtual (n, blk) rows.

Numerics validated by simulation: rel ~5e-3 (gate 2e-2).
"""

import os
from contextlib import ExitStack

import numpy as np
import ml_dtypes

import concourse.bass as bass
import concourse.tile as tile
from concourse import bacc, mybir
from concourse.bass_utils import run_bass_kernel_spmd
from concourse.masks import make_identity

F32 = mybir.dt.float32
BF16 = mybir.dt.bfloat16
E3 = mybir.dt.float8e3
E4 = mybir.dt.float8e4
AF = mybir.ActivationFunctionType
OP = mybir.AluOpType

N, T, D, H = 128, 64, 512, 1024
L = 196
NCORES = 8
NL = N // NCORES          # 16 samples per core
HS = 128                  # kept for test.py's empty-kernel shape
KC = 8                    # h chunks of 128
SCALE = 1.0 / float(np.sqrt(H))

TSTEPS = int(os.environ.get("KERNEL_TSTEPS", T))
REPEAT = int(os.environ.get("KERNEL_REPEAT", "1"))


def _ap(t, dims, offset=0):
    a = t[:]
    return bass.AP(a.tensor, a.offset + offset, [a.ap[0]] + dims)


def _app(tsl, dims, offset=0):
    """AP from a tile slice (keeps partition dim of the slice)."""
    return bass.AP(tsl.tensor, tsl.offset + offset, [tsl.ap[0]] + dims)


def build_nc(tsteps):
    nc = bacc.Bacc("TRN2", target_bir_lowering=False, debug=False,
                   num_devices=NCORES)
    d_a1 = nc.dram_tensor("a1", (128, KC * NL * L), E3, kind="ExternalInput").ap()
    d_a2 = nc.dram_tensor("a2", (128, 2 * 128 * 128), E3, kind="ExternalInput").ap()
    d_wh = nc.dram_tensor("wh", (128, KC * 4096), BF16, kind="ExternalInput").ap()
    d_wa = nc.dram_tensor("wa", (128, KC * 4096), E4, kind="ExternalInput").ap()
    d_xwb = nc.dram_tensor("xwb", (tsteps, 128, 1024), BF16,
                           kind="ExternalInput").ap()
    d_h0t = nc.dram_tensor("h0t", (128, 128), F32, kind="ExternalInput").ap()
    d_repl = nc.dram_tensor("repl", (128, 128), F32, kind="ExternalInput").ap()
    d_sm = nc.dram_tensor("smask", (128, 2 * L), F32, kind="ExternalInput").ap()
    d_rm = nc.dram_tensor("rmask", (128, 512), F32, kind="ExternalInput").ap()
    d_out = nc.dram_tensor("hout", (tsteps, 128, 128), BF16,
                           kind="ExternalOutput").ap()

    with tile.TileContext(nc) as tc:
        with ExitStack() as ctx:
            _build(ctx, tc, tsteps, d_a1, d_a2, d_wh, d_wa, d_xwb, d_h0t,
                   d_repl, d_sm, d_rm, d_out)
    nc.compile()
    return nc


def _build(ctx, tc, tsteps, d_a1, d_a2, d_wh, d_wa, d_xwb, d_h0t, d_repl,
           d_sm, d_rm, d_out):
    nc = tc.nc
    pp = ctx.enter_context(tc.tile_pool(name="persist", bufs=1))
    sb = ctx.enter_context(tc.tile_pool(name="work", bufs=2))
    sx = ctx.enter_context(tc.tile_pool(name="xwb", bufs=2))
    ps_g = ctx.enter_context(tc.tile_pool(name="ps_g", bufs=1, space="PSUM"))
    ps_a = ctx.enter_context(tc.tile_pool(name="ps_a", bufs=1, space="PSUM"))
    ps_s = ctx.enter_context(tc.tile_pool(name="ps_s", bufs=1, space="PSUM"))
    ps_r = ctx.enter_context(tc.tile_pool(name="ps_r", bufs=1, space="PSUM"))
    ps_w = ctx.enter_context(tc.tile_pool(name="ps_w", bufs=1, space="PSUM"))

    # ---- persistent tiles
    t_a1 = pp.tile([128, KC * NL * L], E3)
    t_a2 = pp.tile([128, 2 * 128 * 128], E3)
    t_wh = pp.tile([128, KC * 4096], BF16)
    t_wa = pp.tile([128, KC * 4096], E4)
    t_sm = pp.tile([128, 2 * L], F32)
    t_rm = pp.tile([128, 512], F32)
    t_repl = pp.tile([128, 128], F32)
    t_hbd = pp.tile([128, 2048], BF16)   # scores lhsT block-diag slabs
    t_wbd = pp.tile([128, 2048], BF16)   # readout lhsT block-diag slabs
    t_hT = pp.tile([128, 128], BF16)     # h^T compact (k, n) = gates lhsT
    t_zTa = pp.tile([128, 128], BF16)    # attn^T compact (blk, n)
    t_cT = pp.tile([128, 128], F32)      # c^T state
    t_id = pp.tile([128, 128], BF16)

    # ---- loads + one-time init
    nc.sync.dma_start(t_a1[:], d_a1)
    nc.sync.dma_start(t_a2[:], d_a2)
    nc.sync.dma_start(t_wh[:], d_wh)
    nc.sync.dma_start(t_wa[:], d_wa)
    nc.sync.dma_start(t_sm[:], d_sm)
    nc.sync.dma_start(t_rm[:], d_rm)
    nc.sync.dma_start(t_repl[:], d_repl)
    h0t = pp.tile([128, 128], F32)
    nc.sync.dma_start(h0t[:], d_h0t)
    make_identity(nc, t_id[:])
    nc.vector.memset(t_hbd[:], 0.0)
    nc.vector.memset(t_wbd[:], 0.0)

    g_ps = ps_g.tile([128, 1024], F32)
    aT_ps = ps_a.tile([128, KC * 128], BF16)
    s_ps = ps_s.tile([128, 512], F32)
    r_ps = ps_r.tile([128, 512], F32)
    wT_ps = ps_w.tile([128, 256], BF16, tag="wT")
    attnT_ps = ps_w.tile([128, 128], BF16, tag="aT")
    recv_ps = ps_w.tile([128, 8], F32, tag="rv")
    nc.vector.memset(g_ps[:], 0.0)
    nc.vector.memset(s_ps[:], 0.0)

    nc.vector.tensor_copy(t_cT[:], h0t[:])
    nc.vector.tensor_copy(t_hT[:], h0t[:])

    def fill_hbd():
        """t_hbd slab (q, c) at cols 32*(8q+c); sample n=2q'+... of group q at
        slab col 2(q%2) + (n%2). src = t_hT cols 16c + n.
        Per chunk c: iterate (q2, b, j): q = 2*q2 + b, n = 2q + j:
          dst col = 512*q2 + 258*b + 32*c + j   (258 = 8*32 + 2)
          src col = 16*c + 4*q2 + 2*b + j
        """
        for c in range(KC):
            src = _ap(t_hT, [[4, 4], [2, 2], [1, 2]], offset=16 * c)
            dst = _ap(t_hbd, [[512, 4], [258, 2], [1, 2]], offset=32 * c)
            nc.vector.tensor_copy(dst, src)

    def fill_wbd():
        """t_wbd slab for group g=2n+bh at cols lc*1024 + 32g, lanes at
        colpos 4*(g%8)+lane. dst col = lc*1024 + 256*n2 + 72*r + 36*bh + lane
        (n = 4*n2 + r). src = wT_ps col 32*n2 + r (stride-0 over bh, lane)."""
        for lc in range(2):
            rows = 128 if lc == 0 else 68
            srcsl = wT_ps[0:rows, 128 * lc:128 * (lc + 1)]
            src = _app(srcsl, [[32, 4], [1, 4], [0, 2], [0, 4]])
            dstsl = t_wbd[0:rows, 1024 * lc:1024 * (lc + 1)]
            dst = _app(dstsl, [[256, 4], [72, 4], [36, 2], [1, 4]])
            nc.vector.tensor_copy(dst, src)

    def scores_mms():
        for jc in range(4):
            for b in range(2):
                q = 2 * jc + b
                for c in range(KC):
                    nc.tensor.matmul(
                        s_ps[32 * jc:32 * jc + 32, 0:2 * L],
                        t_hbd[:, 32 * (8 * q + c):32 * (8 * q + c) + 32],
                        t_a1[:, (c * NL + 2 * q) * L:(c * NL + 2 * q + 2) * L],
                        start=(b == 0 and c == 0), stop=(b == 1 and c == KC - 1),
                        tile_position=(0, 32 * jc))

    def readout_mms():
        for a in range(4):
            for gg in range(8):
                g = 8 * a + gg
                for lc in range(2):
                    nc.tensor.matmul(
                        r_ps[32 * a:32 * a + 32, :],
                        t_wbd[:, 1024 * lc + 32 * g:1024 * lc + 32 * g + 32],
                        t_a2[:, (128 * lc + 4 * g) * 128:(128 * lc + 4 * g + 4) * 128],
                        start=(gg == 0 and lc == 0), stop=(gg == 7 and lc == 1),
                        tile_position=(0, 32 * a))

    def gates_mms(xw):
        for k in range(KC):
            for j in range(4):
                for h2 in range(2):
                    nc.tensor.matmul(
                        g_ps[32 * j:32 * j + 16, 512 * h2:512 * h2 + 512],
                        t_hT[:, 16 * k:16 * k + 16],
                        t_wh[:, k * 4096 + 1024 * j + 512 * h2:
                             k * 4096 + 1024 * j + 512 * h2 + 512],
                        start=(k == 0), stop=False, tile_position=(0, 32 * j))
        for k in range(KC):
            for j in range(4):
                for h2 in range(2):
                    nc.tensor.matmul(
                        g_ps[32 * j:32 * j + 16, 512 * h2:512 * h2 + 512],
                        t_zTa[:, 16 * k:16 * k + 16],
                        t_wa[:, k * 4096 + 1024 * j + 512 * h2:
                             k * 4096 + 1024 * j + 512 * h2 + 512],
                        start=False, stop=False, tile_position=(0, 32 * j))
        for j in range(4):
            for h2 in range(2):
                nc.tensor.matmul(
                    g_ps[32 * j:32 * j + 16, 512 * h2:512 * h2 + 512],
                    t_id[32 * j:32 * j + 16, 32 * j:32 * j + 16],
                    xw[32 * j:32 * j + 16, 512 * h2:512 * h2 + 512],
                    start=False, stop=True, tile_position=(32 * j, 32 * j))

    fill_hbd()

    for rep in range(REPEAT):
     for t in range(tsteps):
        # ---- xwb prefetch
        xw = sx.tile([128, 1024], BF16, tag="xw")
        nc.sync.dma_start(xw[:], d_xwb[t])

        # ---- scores (uses t_hbd from previous step's h)
        scores_mms()
        stmp = sb.tile([128, 2 * L], F32, tag="stmp")
        nc.vector.tensor_tensor(stmp[:], s_ps[:, 0:2 * L], t_sm[:], op=OP.mult)
        sc = sb.tile([128, L], F32, tag="sc")
        nc.vector.tensor_reduce(sc[:], _ap(stmp, [[1, L], [L, 2]]),
                                axis=mybir.AxisListType.X, op=OP.add)
        # ---- softmax (max-subtracted, unnormalized; psum holds 2*s)
        m = sb.tile([128, 1], F32, tag="m")
        nc.vector.tensor_reduce(m[:], sc[:], axis=mybir.AxisListType.X, op=OP.max)
        nb = sb.tile([128, 1], F32, tag="nb")
        nc.vector.tensor_scalar_mul(nb[:], m[:], -SCALE / 2.0)
        wexp = sb.tile([128, L], BF16, tag="wexp")
        esum = sb.tile([128, 1], F32, tag="esum")
        nc.scalar.activation(wexp[:], sc[:], AF.Exp, bias=nb[:], scale=SCALE / 2.0,
                             accum_out=esum[:])
        rec = sb.tile([128, 1], F32, tag="rec")
        nc.vector.reciprocal(rec[:], esum[:])
        # rec_v: replicate per-sample 0.5/esum to the 128 virtual rows
        nc.tensor.matmul(recv_ps[:, 0:1], t_repl[:], rec[:], start=True,
                         stop=True)
        recv = sb.tile([128, 1], F32, tag="recv")
        nc.vector.tensor_copy(recv[:], recv_ps[:, 0:1])
        # ---- w^T transposes + readout lhsT fill
        nc.tensor.transpose(wT_ps[:, 0:128], wexp[:, 0:128], t_id[:])
        nc.tensor.transpose(wT_ps[0:68, 128:256], wexp[:, 128:L], t_id[:])
        fill_wbd()
        # ---- readout -> attn
        readout_mms()
        rtmp = sb.tile([128, 512], F32, tag="rtmp")
        nc.vector.tensor_tensor(rtmp[:], r_ps[:], t_rm[:], op=OP.mult)
        attnf = sb.tile([128, 128], F32, tag="attnf")
        nc.vector.tensor_reduce(attnf[:], _ap(rtmp, [[1, 128], [128, 4]]),
                                axis=mybir.AxisListType.X, op=OP.add)
        attnb = sb.tile([128, 128], BF16, tag="attnb")
        nc.vector.tensor_scalar_mul(attnb[:], attnf[:], recv[:])
        nc.tensor.transpose(attnT_ps[:], attnb[:], t_id[:])
        # zTa compact (blk, n): dst col 16*blk + n <- src col 8*n + blk
        nc.vector.tensor_copy(_ap(t_zTa, [[16, 8], [1, 16]]),
                              _app(attnT_ps[:], [[1, 8], [8, 16]]))
        # ---- gates
        gates_mms(xw)
        # ---- pointwise in transposed layout
        g_sb = sb.tile([128, 1024], BF16, tag="g_sb")
        nc.scalar.activation(g_sb[96:112, :], g_ps[96:112, :], AF.Tanh)
        nc.scalar.activation(g_sb[0:80, :], g_ps[0:80, :], AF.Sigmoid)
        for k in range(KC):
            nc.tensor.transpose(aT_ps[:, 128 * k:128 * (k + 1)],
                                g_sb[:, 128 * k:128 * (k + 1)], t_id[:])
        # strided views into aT_ps: quarter q of chunk k at cols 128k+32q..+16
        def quarter(q):
            return _app(aT_ps[:], [[128, KC], [1, 16]], offset=32 * q)
        gTs = sb.tile([128, 128], BF16, tag="gTs")
        nc.vector.tensor_copy(gTs[:], quarter(3))
        c1 = sb.tile([128, 128], F32, tag="c1")
        nc.vector.tensor_tensor(c1[:], quarter(1), t_cT[:], op=OP.mult)
        c2 = sb.tile([128, 128], F32, tag="c2")
        nc.vector.tensor_tensor(c2[:], quarter(0), gTs[:], op=OP.mult)
        nc.vector.tensor_add(t_cT[:], c1[:], c2[:])
        tch = sb.tile([128, 128], F32, tag="tch")
        nc.scalar.activation(tch[:], t_cT[:], AF.Tanh)
        nc.vector.tensor_tensor(t_hT[:], quarter(2), tch[:], op=OP.mult)
        nc.scalar.dma_start(d_out[t], t_hT[:])
        if t < tsteps - 1 or rep < REPEAT - 1:
            fill_hbd()


# ---------------------------------------------------------------------------
# host side
# ---------------------------------------------------------------------------
_NC_CACHE = {}


def _get_nc(tsteps):
    if tsteps not in _NC_CACHE:
        _NC_CACHE[tsteps] = build_nc(tsteps)
    return _NC_CACHE[tsteps]


def _bf(v):
    return v.astype(ml_dtypes.bfloat16)


def prepare_inputs(x, A, Wx, Wh, Wattn, b, tsteps):
    x = np.asarray(x, np.float32)
    A = np.asarray(A, np.float32)
    Wh = np.asarray(Wh, np.float32)
    Wattn = np.asarray(Wattn, np.float32)
    xwb_full = (_bf(x.reshape(N * T, D)).astype(np.float32)
                @ _bf(np.asarray(Wx, np.float32)).astype(np.float32)
                ).reshape(N, T, 4 * H) + np.asarray(b, np.float32)[None, None, :]

    WHs = np.ascontiguousarray(
        _bf(Wh.reshape(KC, 128, 4096).transpose(1, 0, 2)).reshape(128, -1))
    WAs = np.ascontiguousarray(
        Wattn.reshape(KC, 128, 4096).transpose(1, 0, 2)
        .astype(ml_dtypes.float8_e4m3).reshape(128, -1))

    smask = np.zeros((128, 2 * L), np.float32)
    for r in range(128):
        smask[r, (r % 2) * L:(r % 2) * L + L] = 1.0
    rmask = np.zeros((128, 512), np.float32)
    for v in range(128):
        rmask[v, (v % 4) * 128:(v % 4 + 1) * 128] = 1.0
    repl = np.zeros((128, 128), np.float32)
    for n in range(NL):
        for blk in range(KC):
            repl[32 * (n // 4) + n % 4, 8 * n + blk] = 0.5

    in_maps = []
    for k in range(NCORES):
        s0 = NL * k
        Af = A[s0:s0 + NL].reshape(NL, H, L)
        T1 = Af.reshape(NL, KC, 128, L)                      # [n, blk, hh, l]
        a1 = (2.0 * T1.transpose(2, 1, 0, 3)).astype(
            ml_dtypes.float8_e3m4).reshape(128, -1)          # [hh, c, n, l]
        Afp = np.zeros((NL, KC, 128, 256), np.float32)
        Afp[..., :L] = T1
        a2 = (2.0 * Afp.reshape(NL, KC, 128, 2, 128)
              .transpose(4, 3, 0, 1, 2)).astype(
            ml_dtypes.float8_e3m4).reshape(128, -1)          # [lp, lc, n, blk, hh]
        xs = xwb_full[s0:s0 + NL, :tsteps].transpose(1, 0, 2)  # [t, n, 4096]
        xsc = np.zeros((tsteps, 4, 32, 1024), np.float32)
        xsc[:, :, :NL, :] = xs.reshape(tsteps, NL, 4, 1024).transpose(0, 2, 1, 3)
        h0 = Af.mean(-1)                                     # [n, 1024]
        h0t = np.ascontiguousarray(
            h0.reshape(NL, KC, 128).transpose(2, 1, 0).reshape(128, 128))
        in_maps.append({
            "a1": np.ascontiguousarray(a1),
            "a2": np.ascontiguousarray(a2),
            "wh": WHs,
            "wa": WAs,
            "xwb": _bf(xsc.reshape(tsteps, 128, 1024)),
            "h0t": h0t.astype(np.float32),
            "repl": repl,
            "smask": smask,
            "rmask": rmask,
        })
    return in_maps


def kernel(x, A, Wx, Wh, Wattn, b, _tsteps=None):
    tsteps = _tsteps or TSTEPS
    nc = _get_nc(tsteps)
    in_maps = prepare_inputs(x, A, Wx, Wh, Wattn, b, tsteps)
    res = run_bass_kernel_spmd(nc, in_maps, core_ids=list(range(NCORES)))
    out = np.empty((N, tsteps, H), np.float32)
    for k in range(NCORES):
        ho = res.results[k]["hout"].astype(np.float32)       # [t, hh, (kk,n)]
        out[NL * k:NL * (k + 1)] = (
            ho.reshape(tsteps, 128, KC, NL).transpose(3, 0, 2, 1)
            .reshape(NL, tsteps, H))
    if tsteps == T:
        return out
    full = np.zeros((N, T, H), np.float32)
    full[:, :tsteps] = out
    return full


# revision 6
# speedup vs baseline: 5.0989x; 5.0989x over previous
"""Trainium2 Bass kernel for nn_CaptioningRNN (attention LSTM, T=64 steps).

Strategy: PURE DATA-PARALLEL over N (16 samples/core, ZERO collectives).
The baseline TP design paid 2 serialized AllGathers per step (~100-190us/step
of collective latency); here every core runs its 16 samples' full recurrence
independently and only the final output is gathered on the host.

Per-core residents (SBUF, per-partition budget 192KB):
  A1  [hh, (chunk c, n, l)]      E3M4 (stores 2A)   24.5KB  - scores rhs
  A2  [lp, (lc, v=8n+blk, hh)]   E3M4 (stores 2A)   32KB    - readout rhs
  Wh  [p,  (k, 4096)]            bf16               64KB    - gates rhs
  Wa  [p,  (k, 4096)]            E4M3               32KB    - gates rhs
Mixed-dtype matmuls (bf16 lhsT x fp8 rhs) are verified exact on HW, so all
lhsT operands (h^T, exp-weights, attn^T) stay bf16.

Gates: out[16, 4096] via 4x PE column tiling (tile j = gate quarter j at
psum rows 32j..32j+16), 17 chunks each (16 z-chunks + xwb-via-identity).
LSTM pointwise runs in TRANSPOSED layout [128 h, (k, n)]: the four gate
quarters are PE-transposed per 128-chunk, which sidesteps the DVE
equal-partition-base restriction and directly yields h^T for the next
step's lhsT. Softmax is max-subtracted; 1/sum is applied post-readout via
a 0/1 replication matmul that broadcasts the per-sample reciprocal to the
128 virtual (n, blk) rows.

Numerics validated by simulation: rel ~5e-3 (gate 2e-2).
"""

import os
from contextlib import ExitStack

import numpy as np
import ml_dtypes

import concourse.bass as bass
import concourse.tile as tile
from concourse import bacc, mybir
from concourse.bass_utils import run_bass_kernel_spmd
from concourse.masks import make_identity

F32 = mybir.dt.float32
BF16 = mybir.dt.bfloat16
E3 = mybir.dt.float8e3
E4 = mybir.dt.float8e4
AF = mybir.ActivationFunctionType
OP = mybir.AluOpType

N, T, D, H = 128, 64, 512, 1024
L = 196
NCORES = 8
NL = N // NCORES          # 16 samples per core
HS = 128                  # kept for test.py's empty-kernel shape
KC = 8                    # h chunks of 128
SCALE = 1.0 / float(np.sqrt(H))

TSTEPS = int(os.environ.get("KERNEL_TSTEPS", T))


def _ap(t, dims, offset=0):
    a = t[:]
    return bass.AP(a.tensor, a.offset + offset, [a.ap[0]] + dims)


def _app(tsl, dims, offset=0):
    """AP from a tile slice (keeps partition dim of the slice)."""
    return bass.AP(tsl.tensor, tsl.offset + offset, [tsl.ap[0]] + dims)


def build_nc(tsteps, repeat=1):
    nc = bacc.Bacc("TRN2", target_bir_lowering=False, debug=False,
                   num_devices=NCORES)
    d_a1 = nc.dram_tensor("a1", (128, KC * NL * L), E3, kind="ExternalInput").ap()
    d_a2 = nc.dram_tensor("a2", (128, 2 * 128 * 128), E3, kind="ExternalInput").ap()
    d_wh = nc.dram_tensor("wh", (128, KC * 4096), BF16, kind="ExternalInput").ap()
    d_wa = nc.dram_tensor("wa", (128, KC * 4096), E4, kind="ExternalInput").ap()
    d_xwb = nc.dram_tensor("xwb", (tsteps, 128, 1024), BF16,
                           kind="ExternalInput").ap()
    d_h0t = nc.dram_tensor("h0t", (128, 128), F32, kind="ExternalInput").ap()
    d_repl = nc.dram_tensor("repl", (128, 128), F32, kind="ExternalInput").ap()
    d_sm = nc.dram_tensor("smask", (128, 2 * L), F32, kind="ExternalInput").ap()
    d_rm = nc.dram_tensor("rmask", (128, 512), F32, kind="ExternalInput").ap()
    d_out = nc.dram_tensor("hout", (tsteps, 128, 128), BF16,
                           kind="ExternalOutput").ap()

    with tile.TileContext(nc) as tc:
        with ExitStack() as ctx:
            _build(ctx, tc, tsteps, d_a1, d_a2, d_wh, d_wa, d_xwb, d_h0t,
                   d_repl, d_sm, d_rm, d_out, repeat)
    nc.compile()
    return nc


def _build(ctx, tc, tsteps, d_a1, d_a2, d_wh, d_wa, d_xwb, d_h0t, d_repl,
           d_sm, d_rm, d_out, repeat=1):
    nc = tc.nc
    pp = ctx.enter_context(tc.tile_pool(name="persist", bufs=1))
    sb = ctx.enter_context(tc.tile_pool(name="work", bufs=2))
    sx = ctx.enter_context(tc.tile_pool(name="xwb", bufs=2))
    ps_g = ctx.enter_context(tc.tile_pool(name="ps_g", bufs=1, space="PSUM"))
    ps_a = ctx.enter_context(tc.tile_pool(name="ps_a", bufs=1, space="PSUM"))
    ps_s = ctx.enter_context(tc.tile_pool(name="ps_s", bufs=1, space="PSUM"))
    ps_r = ctx.enter_context(tc.tile_pool(name="ps_r", bufs=1, space="PSUM"))
    ps_w = ctx.enter_context(tc.tile_pool(name="ps_w", bufs=1, space="PSUM"))

    # ---- persistent tiles
    t_a1 = pp.tile([128, KC * NL * L], E3)
    t_a2 = pp.tile([128, 2 * 128 * 128], E3)
    t_wh = pp.tile([128, KC * 4096], BF16)
    t_wa = pp.tile([128, KC * 4096], E4)
    t_sm = pp.tile([128, 2 * L], F32)
    t_rm = pp.tile([128, 512], F32)
    t_repl = pp.tile([128, 128], F32)
    t_hbd = pp.tile([128, 2048], BF16)   # scores lhsT block-diag slabs
    t_wbd = pp.tile([128, 2048], BF16)   # readout lhsT block-diag slabs
    t_hT = pp.tile([128, 128], BF16)     # h^T compact (k, n) = gates lhsT
    t_zTa = pp.tile([128, 128], BF16)    # attn^T compact (blk, n)
    t_cT = pp.tile([128, 128], F32)      # c^T state
    t_id = pp.tile([128, 128], BF16)

    # ---- loads + one-time init
    nc.sync.dma_start(t_a1[:], d_a1)
    nc.sync.dma_start(t_a2[:], d_a2)
    nc.sync.dma_start(t_wh[:], d_wh)
    nc.sync.dma_start(t_wa[:], d_wa)
    nc.sync.dma_start(t_sm[:], d_sm)
    nc.sync.dma_start(t_rm[:], d_rm)
    nc.sync.dma_start(t_repl[:], d_repl)
    h0t = pp.tile([128, 128], F32)
    nc.sync.dma_start(h0t[:], d_h0t)
    make_identity(nc, t_id[:])
    nc.vector.memset(t_hbd[:], 0.0)
    nc.vector.memset(t_wbd[:], 0.0)

    g_ps = ps_g.tile([128, 1024], F32)
    aT_ps = ps_a.tile([128, KC * 128], BF16)
    s_ps = ps_s.tile([128, 512], F32)
    r_ps = ps_r.tile([128, 512], F32)
    wT_ps = ps_w.tile([128, 256], BF16, tag="wT")
    attnT_ps = ps_w.tile([128, 128], BF16, tag="aT")
    recv_ps = ps_w.tile([128, 8], F32, tag="rv")
    nc.vector.memset(g_ps[:], 0.0)
    nc.vector.memset(s_ps[:], 0.0)

    nc.vector.tensor_copy(t_cT[:], h0t[:])
    nc.vector.tensor_copy(t_hT[:], h0t[:])

    def fill_hbd():
        """t_hbd slab (q, c) at cols 32*(8q+c); sample n=2q'+... of group q at
        slab col 2(q%2) + (n%2). src = t_hT cols 16c + n.
        Per chunk c: iterate (q2, b, j): q = 2*q2 + b, n = 2q + j:
          dst col = 512*q2 + 258*b + 32*c + j   (258 = 8*32 + 2)
          src col = 16*c + 4*q2 + 2*b + j
        """
        for c in range(KC):
            src = _ap(t_hT, [[4, 4], [2, 2], [1, 2]], offset=16 * c)
            dst = _ap(t_hbd, [[512, 4], [258, 2], [1, 2]], offset=32 * c)
            nc.vector.tensor_copy(dst, src)

    def fill_wbd():
        """t_wbd slab for group g=2n+bh at cols lc*1024 + 32g, lanes at
        colpos 4*(g%8)+lane. dst col = lc*1024 + 256*n2 + 72*r + 36*bh + lane
        (n = 4*n2 + r). src = wT_ps col 32*n2 + r (stride-0 over bh, lane)."""
        for lc in range(2):
            rows = 128 if lc == 0 else 68
            srcsl = wT_ps[0:rows, 128 * lc:128 * (lc + 1)]
            src = _app(srcsl, [[32, 4], [1, 4], [0, 2], [0, 4]])
            dstsl = t_wbd[0:rows, 1024 * lc:1024 * (lc + 1)]
            dst = _app(dstsl, [[256, 4], [72, 4], [36, 2], [1, 4]])
            nc.vector.tensor_copy(dst, src)

    def scores_mms():
        for jc in range(4):
            for b in range(2):
                q = 2 * jc + b
                for c in range(KC):
                    nc.tensor.matmul(
                        s_ps[32 * jc:32 * jc + 32, 0:2 * L],
                        t_hbd[:, 32 * (8 * q + c):32 * (8 * q + c) + 32],
                        t_a1[:, (c * NL + 2 * q) * L:(c * NL + 2 * q + 2) * L],
                        start=(b == 0 and c == 0), stop=(b == 1 and c == KC - 1),
                        tile_position=(0, 32 * jc))

    def readout_mms():
        for a in range(4):
            for gg in range(8):
                g = 8 * a + gg
                for lc in range(2):
                    nc.tensor.matmul(
                        r_ps[32 * a:32 * a + 32, :],
                        t_wbd[:, 1024 * lc + 32 * g:1024 * lc + 32 * g + 32],
                        t_a2[:, (128 * lc + 4 * g) * 128:(128 * lc + 4 * g + 4) * 128],
                        start=(gg == 0 and lc == 0), stop=(gg == 7 and lc == 1),
                        tile_position=(0, 32 * a))

    def gates_mms(xw):
        for k in range(KC):
            for j in range(4):
                for h2 in range(2):
                    nc.tensor.matmul(
                        g_ps[32 * j:32 * j + 16, 512 * h2:512 * h2 + 512],
                        t_hT[:, 16 * k:16 * k + 16],
                        t_wh[:, k * 4096 + 1024 * j + 512 * h2:
                             k * 4096 + 1024 * j + 512 * h2 + 512],
                        start=(k == 0), stop=False, tile_position=(0, 32 * j))
        for k in range(KC):
            for j in range(4):
                for h2 in range(2):
                    nc.tensor.matmul(
                        g_ps[32 * j:32 * j + 16, 512 * h2:512 * h2 + 512],
                        t_zTa[:, 16 * k:16 * k + 16],
                        t_wa[:, k * 4096 + 1024 * j + 512 * h2:
                             k * 4096 + 1024 * j + 512 * h2 + 512],
                        start=False, stop=False, tile_position=(0, 32 * j))
        for j in range(4):
            for h2 in range(2):
                nc.tensor.matmul(
                    g_ps[32 * j:32 * j + 16, 512 * h2:512 * h2 + 512],
                    t_id[32 * j:32 * j + 16, 32 * j:32 * j + 16],
                    xw[32 * j:32 * j + 16, 512 * h2:512 * h2 + 512],
                    start=False, stop=True, tile_position=(32 * j, 32 * j))

    fill_hbd()

    for rep in range(repeat):
     for t in range(tsteps):
        # ---- xwb prefetch
        xw = sx.tile([128, 1024], BF16, tag="xw")
        nc.sync.dma_start(xw[:], d_xwb[t])

        # ---- scores (uses t_hbd from previous step's h)
        scores_mms()
        stmp = sb.tile([128, 2 * L], F32, tag="stmp")
        nc.vector.tensor_tensor(stmp[:], s_ps[:, 0:2 * L], t_sm[:], op=OP.mult)
        sc = sb.tile([128, L], F32, tag="sc")
        nc.vector.tensor_reduce(sc[:], _ap(stmp, [[1, L], [L, 2]]),
                                axis=mybir.AxisListType.X, op=OP.add)
        # ---- softmax (max-subtracted, unnormalized; psum holds 2*s)
        m = sb.tile([128, 1], F32, tag="m")
        nc.vector.tensor_reduce(m[:], sc[:], axis=mybir.AxisListType.X, op=OP.max)
        nb = sb.tile([128, 1], F32, tag="nb")
        nc.vector.tensor_scalar_mul(nb[:], m[:], -SCALE / 2.0)
        wexp = sb.tile([128, L], BF16, tag="wexp")
        esum = sb.tile([128, 1], F32, tag="esum")
        nc.scalar.activation(wexp[:], sc[:], AF.Exp, bias=nb[:], scale=SCALE / 2.0,
                             accum_out=esum[:])
        rec = sb.tile([128, 1], F32, tag="rec")
        nc.vector.reciprocal(rec[:], esum[:])
        # rec_v: replicate per-sample 0.5/esum to the 128 virtual rows
        nc.tensor.matmul(recv_ps[:, 0:1], t_repl[:], rec[:], start=True,
                         stop=True)
        recv = sb.tile([128, 1], F32, tag="recv")
        nc.vector.tensor_copy(recv[:], recv_ps[:, 0:1])
        # ---- w^T transposes + readout lhsT fill
        nc.tensor.transpose(wT_ps[:, 0:128], wexp[:, 0:128], t_id[:])
        nc.tensor.transpose(wT_ps[0:68, 128:256], wexp[:, 128:L], t_id[:])
        fill_wbd()
        # ---- readout -> attn
        readout_mms()
        rtmp = sb.tile([128, 512], F32, tag="rtmp")
        nc.vector.tensor_tensor(rtmp[:], r_ps[:], t_rm[:], op=OP.mult)
        attnf = sb.tile([128, 128], F32, tag="attnf")
        nc.vector.tensor_reduce(attnf[:], _ap(rtmp, [[1, 128], [128, 4]]),
                                axis=mybir.AxisListType.X, op=OP.add)
        attnb = sb.tile([128, 128], BF16, tag="attnb")
        nc.vector.tensor_scalar_mul(attnb[:], attnf[:], recv[:])
        nc.tensor.transpose(attnT_ps[:], attnb[:], t_id[:])
        # zTa compact (blk, n): dst col 16*blk + n <- src col 8*n + blk
        nc.vector.tensor_copy(_ap(t_zTa, [[16, 8], [1, 16]]),
                              _app(attnT_ps[:], [[1, 8], [8, 16]]))
        # ---- gates
        gates_mms(xw)
        # ---- pointwise in transposed layout
        g_sb = sb.tile([128, 1024], BF16, tag="g_sb")
        nc.scalar.activation(g_sb[96:112, :], g_ps[96:112, :], AF.Tanh)
        nc.scalar.activation(g_sb[0:80, :], g_ps[0:80, :], AF.Sigmoid)
        for k in range(KC):
            nc.tensor.transpose(aT_ps[:, 128 * k:128 * (k + 1)],
                                g_sb[:, 128 * k:128 * (k + 1)], t_id[:])
        # strided views into aT_ps: quarter q of chunk k at cols 128k+32q..+16
        def quarter(q):
            return _app(aT_ps[:], [[128, KC], [1, 16]], offset=32 * q)
        gTs = sb.tile([128, 128], BF16, tag="gTs")
        nc.vector.tensor_copy(gTs[:], quarter(3))
        c1 = sb.tile([128, 128], F32, tag="c1")
        nc.vector.tensor_tensor(c1[:], quarter(1), t_cT[:], op=OP.mult)
        c2 = sb.tile([128, 128], F32, tag="c2")
        nc.vector.tensor_tensor(c2[:], quarter(0), gTs[:], op=OP.mult)
        nc.vector.tensor_add(t_cT[:], c1[:], c2[:])
        tch = sb.tile([128, 128], F32, tag="tch")
        nc.scalar.activation(tch[:], t_cT[:], AF.Tanh)
        nc.vector.tensor_tensor(t_hT[:], quarter(2), tch[:], op=OP.mult)
        nc.scalar.dma_start(d_out[t], t_hT[:])
        if t < tsteps - 1 or rep < repeat - 1:
            fill_hbd()


# ---------------------------------------------------------------------------
# host side
# ---------------------------------------------------------------------------
_NC_CACHE = {}


def _get_nc(tsteps, repeat=1):
    key = (tsteps, repeat)
    if key not in _NC_CACHE:
        _NC_CACHE[key] = build_nc(tsteps, repeat)
    return _NC_CACHE[key]


def _bf(v):
    return v.astype(ml_dtypes.bfloat16)


def prepare_inputs(x, A, Wx, Wh, Wattn, b, tsteps):
    x = np.asarray(x, np.float32)
    A = np.asarray(A, np.float32)
    Wh = np.asarray(Wh, np.float32)
    Wattn = np.asarray(Wattn, np.float32)
    xwb_full = (_bf(x.reshape(N * T, D)).astype(np.float32)
                @ _bf(np.asarray(Wx, np.float32)).astype(np.float32)
                ).reshape(N, T, 4 * H) + np.asarray(b, np.float32)[None, None, :]

    WHs = np.ascontiguousarray(
        _bf(Wh.reshape(KC, 128, 4096).transpose(1, 0, 2)).reshape(128, -1))
    WAs = np.ascontiguousarray(
        Wattn.reshape(KC, 128, 4096).transpose(1, 0, 2)
        .astype(ml_dtypes.float8_e4m3).reshape(128, -1))

    smask = np.zeros((128, 2 * L), np.float32)
    for r in range(128):
        smask[r, (r % 2) * L:(r % 2) * L + L] = 1.0
    rmask = np.zeros((128, 512), np.float32)
    for v in range(128):
        rmask[v, (v % 4) * 128:(v % 4 + 1) * 128] = 1.0
    repl = np.zeros((128, 128), np.float32)
    for n in range(NL):
        for blk in range(KC):
            repl[32 * (n // 4) + n % 4, 8 * n + blk] = 0.5

    in_maps = []
    for k in range(NCORES):
        s0 = NL * k
        Af = A[s0:s0 + NL].reshape(NL, H, L)
        T1 = Af.reshape(NL, KC, 128, L)                      # [n, blk, hh, l]
        a1 = (2.0 * T1.transpose(2, 1, 0, 3)).astype(
            ml_dtypes.float8_e3m4).reshape(128, -1)          # [hh, c, n, l]
        Afp = np.zeros((NL, KC, 128, 256), np.float32)
        Afp[..., :L] = T1
        a2 = (2.0 * Afp.reshape(NL, KC, 128, 2, 128)
              .transpose(4, 3, 0, 1, 2)).astype(
            ml_dtypes.float8_e3m4).reshape(128, -1)          # [lp, lc, n, blk, hh]
        xs = xwb_full[s0:s0 + NL, :tsteps].transpose(1, 0, 2)  # [t, n, 4096]
        xsc = np.zeros((tsteps, 4, 32, 1024), np.float32)
        xsc[:, :, :NL, :] = xs.reshape(tsteps, NL, 4, 1024).transpose(0, 2, 1, 3)
        h0 = Af.mean(-1)                                     # [n, 1024]
        h0t = np.ascontiguousarray(
            h0.reshape(NL, KC, 128).transpose(2, 1, 0).reshape(128, 128))
        in_maps.append({
            "a1": np.ascontiguousarray(a1),
            "a2": np.ascontiguousarray(a2),
            "wh": WHs,
            "wa": WAs,
            "xwb": _bf(xsc.reshape(tsteps, 128, 1024)),
            "h0t": h0t.astype(np.float32),
            "repl": repl,
            "smask": smask,
            "rmask": rmask,
        })
    return in_maps


def kernel(x, A, Wx, Wh, Wattn, b, _tsteps=None):
    tsteps = _tsteps or TSTEPS
    nc = _get_nc(tsteps)
    in_maps = prepare_inputs(x, A, Wx, Wh, Wattn, b, tsteps)
    res = run_bass_kernel_spmd(nc, in_maps, core_ids=list(range(NCORES)))
    out = np.empty((N, tsteps, H), np.float32)
    for k in range(NCORES):
        ho = res.results[k]["hout"].astype(np.float32)       # [t, hh, (kk,n)]
        out[NL * k:NL * (k + 1)] = (
            ho.reshape(tsteps, 128, KC, NL).transpose(3, 0, 2, 1)
            .reshape(NL, tsteps, H))
    if tsteps == T:
        return out
    full = np.zeros((N, T, H), np.float32)
    full[:, :tsteps] = out
    return full


# revision 9
# speedup vs baseline: 5.6281x; 1.1038x over previous
"""Trainium2 Bass kernel for nn_CaptioningRNN (attention LSTM, T=64 steps).

Strategy: PURE DATA-PARALLEL over N (16 samples/core, ZERO collectives).
The baseline TP design paid 2 serialized AllGathers per step (~100-190us/step
of collective latency); here every core runs its 16 samples' full recurrence
independently and only the final output is gathered on the host.

Per-core residents (SBUF, per-partition budget 192KB):
  A1  [hh, (chunk c, n, l)]      E3M4 (stores 2A)   24.5KB  - scores rhs
  A2  [lp, (lc, v=8n+blk, hh)]   E3M4 (stores 2A)   32KB    - readout rhs
  Wh  [p,  (k, 4096)]            bf16               64KB    - gates rhs
  Wa  [p,  (k, 4096)]            E4M3               32KB    - gates rhs
Mixed-dtype matmuls (bf16 lhsT x fp8 rhs) are verified exact on HW, so all
lhsT operands (h^T, exp-weights, attn^T) stay bf16.

Gates: out[16, 4096] via 4x PE column tiling (tile j = gate quarter j at
psum rows 32j..32j+16), 17 chunks each (16 z-chunks + xwb-via-identity).
LSTM pointwise runs in TRANSPOSED layout [128 h, (k, n)]: the four gate
quarters are PE-transposed per 128-chunk, which sidesteps the DVE
equal-partition-base restriction and directly yields h^T for the next
step's lhsT. Softmax is max-subtracted; 1/sum is applied post-readout via
a 0/1 replication matmul that broadcasts the per-sample reciprocal to the
128 virtual (n, blk) rows.

Numerics validated by simulation: rel ~5e-3 (gate 2e-2).
"""

import os
from contextlib import ExitStack

import numpy as np
import ml_dtypes

import concourse.bass as bass
import concourse.tile as tile
from concourse import bacc, mybir
from concourse.bass_utils import run_bass_kernel_spmd
from concourse.masks import make_identity

F32 = mybir.dt.float32
BF16 = mybir.dt.bfloat16
E3 = mybir.dt.float8e3
E4 = mybir.dt.float8e4
AF = mybir.ActivationFunctionType
OP = mybir.AluOpType

N, T, D, H = 128, 64, 512, 1024
L = 196
NCORES = 8
NL = N // NCORES          # 16 samples per core
HS = 128                  # kept for test.py's empty-kernel shape
KC = 8                    # h chunks of 128
SCALE = 1.0 / float(np.sqrt(H))

TSTEPS = int(os.environ.get("KERNEL_TSTEPS", T))
ABL = os.environ.get("KERNEL_ABL", "")  # csv: noatt,nogates,nopoint,noscore,noread


def _ap(t, dims, offset=0):
    a = t[:]
    return bass.AP(a.tensor, a.offset + offset, [a.ap[0]] + dims)


def _app(tsl, dims, offset=0):
    """AP from a tile slice (keeps partition dim of the slice)."""
    return bass.AP(tsl.tensor, tsl.offset + offset, [tsl.ap[0]] + dims)


def build_nc(tsteps, repeat=1):
    nc = bacc.Bacc("TRN2", target_bir_lowering=False, debug=False,
                   num_devices=NCORES)
    d_a1 = nc.dram_tensor("a1", (128, KC * NL * L), E3, kind="ExternalInput").ap()
    d_a2 = nc.dram_tensor("a2", (128, 2 * 128 * 128), E3, kind="ExternalInput").ap()
    d_wh = nc.dram_tensor("wh", (128, KC * 4096), BF16, kind="ExternalInput").ap()
    d_wa = nc.dram_tensor("wa", (128, KC * 4096), E4, kind="ExternalInput").ap()
    d_xwb = nc.dram_tensor("xwb", (tsteps, 128, 1024), BF16,
                           kind="ExternalInput").ap()
    d_h0t = nc.dram_tensor("h0t", (128, 128), F32, kind="ExternalInput").ap()
    d_repl = nc.dram_tensor("repl", (128, 128), F32, kind="ExternalInput").ap()
    d_sm = nc.dram_tensor("smask", (128, 2 * L), F32, kind="ExternalInput").ap()
    d_rm = nc.dram_tensor("rmask", (128, 512), F32, kind="ExternalInput").ap()
    d_out = nc.dram_tensor("hout", (tsteps, 128, 128), BF16,
                           kind="ExternalOutput").ap()

    with tile.TileContext(nc) as tc:
        with ExitStack() as ctx:
            _build(ctx, tc, tsteps, d_a1, d_a2, d_wh, d_wa, d_xwb, d_h0t,
                   d_repl, d_sm, d_rm, d_out, repeat)
    nc.compile()
    return nc


def _build(ctx, tc, tsteps, d_a1, d_a2, d_wh, d_wa, d_xwb, d_h0t, d_repl,
           d_sm, d_rm, d_out, repeat=1):
    nc = tc.nc
    pp = ctx.enter_context(tc.tile_pool(name="persist", bufs=1))
    sb = ctx.enter_context(tc.tile_pool(name="work", bufs=2))
    sx = ctx.enter_context(tc.tile_pool(name="xwb", bufs=2))
    ps_g = ctx.enter_context(tc.tile_pool(name="ps_g", bufs=1, space="PSUM"))
    ps_a = ctx.enter_context(tc.tile_pool(name="ps_a", bufs=1, space="PSUM"))
    ps_s = ctx.enter_context(tc.tile_pool(name="ps_s", bufs=1, space="PSUM"))
    ps_r = ctx.enter_context(tc.tile_pool(name="ps_r", bufs=1, space="PSUM"))
    ps_w = ctx.enter_context(tc.tile_pool(name="ps_w", bufs=1, space="PSUM"))

    # ---- persistent tiles
    t_a1 = pp.tile([128, KC * NL * L], E3)
    t_a2 = pp.tile([128, 2 * 128 * 128], E3)
    t_wh = pp.tile([128, KC * 4096], BF16)
    t_wa = pp.tile([128, KC * 4096], E4)
    t_sm = pp.tile([128, 2 * L], F32)
    t_rm = pp.tile([128, 512], F32)
    t_repl = pp.tile([128, 128], F32)
    t_hbd = pp.tile([128, 2048], BF16)   # scores lhsT block-diag slabs
    t_wbd = pp.tile([128, 2048], BF16)   # readout lhsT block-diag slabs
    t_hT = pp.tile([128, 128], BF16)     # h^T compact (k, n) = gates lhsT
    t_zTa = pp.tile([128, 128], BF16)    # attn^T compact (blk, n)
    t_cT = pp.tile([128, 128], F32)      # c^T state
    t_id = pp.tile([128, 128], BF16)

    # ---- loads + one-time init
    nc.sync.dma_start(t_a1[:], d_a1)
    nc.sync.dma_start(t_a2[:], d_a2)
    nc.sync.dma_start(t_wh[:], d_wh)
    nc.sync.dma_start(t_wa[:], d_wa)
    nc.sync.dma_start(t_sm[:], d_sm)
    nc.sync.dma_start(t_rm[:], d_rm)
    nc.sync.dma_start(t_repl[:], d_repl)
    h0t = pp.tile([128, 128], F32)
    nc.sync.dma_start(h0t[:], d_h0t)
    make_identity(nc, t_id[:])
    nc.vector.memset(t_hbd[:], 0.0)
    nc.vector.memset(t_wbd[:], 0.0)

    g_ps = ps_g.tile([128, 1024], F32)
    aT_ps = ps_a.tile([128, KC * 128], BF16)
    s_ps = ps_s.tile([128, 512], F32)
    r_ps = ps_r.tile([128, 512], F32)
    wT_ps = ps_w.tile([128, 256], BF16, tag="wT")
    attnT_ps = ps_w.tile([128, 128], BF16, tag="aT")
    recv_ps = ps_w.tile([128, 8], F32, tag="rv")
    nc.vector.memset(g_ps[:], 0.0)
    nc.vector.memset(s_ps[:], 0.0)

    nc.vector.tensor_copy(t_cT[:], h0t[:])
    nc.vector.tensor_copy(t_hT[:], h0t[:])

    def fill_hbd():
        """t_hbd slab (q, c) at cols 32*(8q+c); sample n=2q'+... of group q at
        slab col 2(q%2) + (n%2). src = t_hT cols 16c + n.
        Per chunk c: iterate (q2, b, j): q = 2*q2 + b, n = 2q + j:
          dst col = 512*q2 + 258*b + 32*c + j   (258 = 8*32 + 2)
          src col = 16*c + 4*q2 + 2*b + j
        """
        for c in range(KC):
            src = _ap(t_hT, [[4, 4], [2, 2], [1, 2]], offset=16 * c)
            dst = _ap(t_hbd, [[512, 4], [258, 2], [1, 2]], offset=32 * c)
            nc.vector.tensor_copy(dst, src)

    def fill_wbd():
        """t_wbd slab for group g=2n+bh at cols lc*1024 + 32g, lanes at
        colpos 4*(g%8)+lane. dst col = lc*1024 + 256*n2 + 72*r + 36*bh + lane
        (n = 4*n2 + r). src = wT_ps col 32*n2 + r (stride-0 over bh, lane)."""
        for lc in range(2):
            rows = 128 if lc == 0 else 68
            srcsl = wT_ps[0:rows, 128 * lc:128 * (lc + 1)]
            src = _app(srcsl, [[32, 4], [1, 4], [0, 2], [0, 4]])
            dstsl = t_wbd[0:rows, 1024 * lc:1024 * (lc + 1)]
            dst = _app(dstsl, [[256, 4], [72, 4], [36, 2], [1, 4]])
            nc.vector.tensor_copy(dst, src)

    def scores_mms():
        for jc in range(4):
            for b in range(2):
                q = 2 * jc + b
                for c in range(KC):
                    nc.tensor.matmul(
                        s_ps[32 * jc:32 * jc + 32, 0:2 * L],
                        t_hbd[:, 32 * (8 * q + c):32 * (8 * q + c) + 32],
                        t_a1[:, (c * NL + 2 * q) * L:(c * NL + 2 * q + 2) * L],
                        start=(b == 0 and c == 0), stop=(b == 1 and c == KC - 1),
                        tile_position=(0, 32 * jc))

    def readout_mms():
        for a in range(4):
            for gg in range(8):
                g = 8 * a + gg
                for lc in range(2):
                    nc.tensor.matmul(
                        r_ps[32 * a:32 * a + 32, :],
                        t_wbd[:, 1024 * lc + 32 * g:1024 * lc + 32 * g + 32],
                        t_a2[:, (128 * lc + 4 * g) * 128:(128 * lc + 4 * g + 4) * 128],
                        start=(gg == 0 and lc == 0), stop=(gg == 7 and lc == 1),
                        tile_position=(0, 32 * a))

    def gates_h(xw):
        for k in range(KC):
            for j in range(4):
                for h2 in range(2):
                    nc.tensor.matmul(
                        g_ps[32 * j:32 * j + 16, 512 * h2:512 * h2 + 512],
                        t_hT[:, 16 * k:16 * k + 16],
                        t_wh[:, k * 4096 + 1024 * j + 512 * h2:
                             k * 4096 + 1024 * j + 512 * h2 + 512],
                        start=(k == 0), stop=False, tile_position=(0, 32 * j))
        for j in range(4):
            for h2 in range(2):
                nc.tensor.matmul(
                    g_ps[32 * j:32 * j + 16, 512 * h2:512 * h2 + 512],
                    t_id[32 * j:32 * j + 16, 32 * j:32 * j + 16],
                    xw[32 * j:32 * j + 16, 512 * h2:512 * h2 + 512],
                    start=False, stop=False, tile_position=(32 * j, 32 * j))

    def gates_attn():
        for k in range(KC):
            for j in range(4):
                for h2 in range(2):
                    nc.tensor.matmul(
                        g_ps[32 * j:32 * j + 16, 512 * h2:512 * h2 + 512],
                        t_zTa[:, 16 * k:16 * k + 16],
                        t_wa[:, k * 4096 + 1024 * j + 512 * h2:
                             k * 4096 + 1024 * j + 512 * h2 + 512],
                        start=False, stop=(k == KC - 1), tile_position=(0, 32 * j))

    def gates_mms(xw):
        gates_h(xw)
        gates_attn()

    def quarter(q):
        return _app(aT_ps[:], [[128, KC], [1, 16]], offset=32 * q)

    def pointwise(t, last):
        g_sb = sb.tile([128, 1024], BF16, tag="g_sb")
        nc.scalar.activation(g_sb[96:112, :], g_ps[96:112, :], AF.Tanh)
        nc.scalar.activation(g_sb[0:80, :], g_ps[0:80, :], AF.Sigmoid)
        for k in range(KC):
            nc.tensor.transpose(aT_ps[:, 128 * k:128 * (k + 1)],
                                g_sb[:, 128 * k:128 * (k + 1)], t_id[:])
        gTs = sb.tile([128, 128], BF16, tag="gTs")
        nc.vector.tensor_copy(gTs[:], quarter(3))
        c1 = sb.tile([128, 128], F32, tag="c1")
        nc.vector.tensor_tensor(c1[:], quarter(1), t_cT[:], op=OP.mult)
        c2 = sb.tile([128, 128], F32, tag="c2")
        nc.vector.tensor_tensor(c2[:], quarter(0), gTs[:], op=OP.mult)
        nc.vector.tensor_add(t_cT[:], c1[:], c2[:])
        tch = sb.tile([128, 128], F32, tag="tch")
        nc.scalar.activation(tch[:], t_cT[:], AF.Tanh)
        nc.vector.tensor_tensor(t_hT[:], quarter(2), tch[:], op=OP.mult)
        nc.scalar.dma_start(d_out[t], t_hT[:])
        if not last:
            fill_hbd()

    fill_hbd()

    for rep in range(repeat):
     for t in range(tsteps):
        last = (t == tsteps - 1 and rep == repeat - 1)
        # ---- xwb prefetch
        xw = sx.tile([128, 1024], BF16, tag="xw")
        nc.sync.dma_start(xw[:], d_xwb[t])

        if "noatt" in ABL:
            gates_mms(xw)
            pointwise(t, last)
            continue
        # ---- scores (uses t_hbd from previous step's h)
        if "noscore" not in ABL:
            scores_mms()
        # ---- gates h-part: PE busy while DVE/ACT run extract+softmax
        if "nogates" not in ABL:
            gates_h(xw)
        stmp = sb.tile([128, 2 * L], F32, tag="stmp")
        nc.vector.tensor_tensor(stmp[:], s_ps[:, 0:2 * L], t_sm[:], op=OP.mult)
        sc = sb.tile([128, L], F32, tag="sc")
        nc.vector.tensor_reduce(sc[:], _ap(stmp, [[1, L], [L, 2]]),
                                axis=mybir.AxisListType.X, op=OP.add)
        # ---- softmax (max-subtracted, unnormalized; psum holds 2*s)
        m = sb.tile([128, 1], F32, tag="m")
        nc.vector.tensor_reduce(m[:], sc[:], axis=mybir.AxisListType.X, op=OP.max)
        nb = sb.tile([128, 1], F32, tag="nb")
        nc.vector.tensor_scalar_mul(nb[:], m[:], -SCALE / 2.0)
        wexp = sb.tile([128, L], BF16, tag="wexp")
        esum = sb.tile([128, 1], F32, tag="esum")
        nc.scalar.activation(wexp[:], sc[:], AF.Exp, bias=nb[:], scale=SCALE / 2.0,
                             accum_out=esum[:])
        rec = sb.tile([128, 1], F32, tag="rec")
        nc.vector.reciprocal(rec[:], esum[:])
        # ---- w^T transposes + rec_v replication (PE, tiny)
        nc.tensor.transpose(wT_ps[:, 0:128], wexp[:, 0:128], t_id[:])
        nc.tensor.transpose(wT_ps[0:68, 128:256], wexp[:, 128:L], t_id[:])
        nc.tensor.matmul(recv_ps[:, 0:1], t_repl[:], rec[:], start=True,
                         stop=True)
        recv = sb.tile([128, 1], F32, tag="recv")
        nc.vector.tensor_copy(recv[:], recv_ps[:, 0:1])
        fill_wbd()
        # ---- readout -> attn
        if "noread" not in ABL:
            readout_mms()
        rtmp = sb.tile([128, 512], F32, tag="rtmp")
        nc.vector.tensor_tensor(rtmp[:], r_ps[:], t_rm[:], op=OP.mult)
        attnf = sb.tile([128, 128], F32, tag="attnf")
        nc.vector.tensor_reduce(attnf[:], _ap(rtmp, [[1, 128], [128, 4]]),
                                axis=mybir.AxisListType.X, op=OP.add)
        attnb = sb.tile([128, 128], BF16, tag="attnb")
        nc.vector.tensor_scalar_mul(attnb[:], attnf[:], recv[:])
        nc.tensor.transpose(attnT_ps[:], attnb[:], t_id[:])
        # zTa compact (blk, n): dst col 16*blk + n <- src col 8*n + blk
        nc.vector.tensor_copy(_ap(t_zTa, [[16, 8], [1, 16]]),
                              _app(attnT_ps[:], [[1, 8], [8, 16]]))
        # ---- gates attn-part
        if "nogates" not in ABL:
            gates_attn()
        pointwise(t, last)

# ---------------------------------------------------------------------------
# host side
# ---------------------------------------------------------------------------
_NC_CACHE = {}


def _get_nc(tsteps, repeat=1):
    key = (tsteps, repeat)
    if key not in _NC_CACHE:
        _NC_CACHE[key] = build_nc(tsteps, repeat)
    return _NC_CACHE[key]


def _bf(v):
    return v.astype(ml_dtypes.bfloat16)


def prepare_inputs(x, A, Wx, Wh, Wattn, b, tsteps):
    x = np.asarray(x, np.float32)
    A = np.asarray(A, np.float32)
    Wh = np.asarray(Wh, np.float32)
    Wattn = np.asarray(Wattn, np.float32)
    xwb_full = (_bf(x.reshape(N * T, D)).astype(np.float32)
                @ _bf(np.asarray(Wx, np.float32)).astype(np.float32)
                ).reshape(N, T, 4 * H) + np.asarray(b, np.float32)[None, None, :]

    WHs = np.ascontiguousarray(
        _bf(Wh.reshape(KC, 128, 4096).transpose(1, 0, 2)).reshape(128, -1))
    WAs = np.ascontiguousarray(
        Wattn.reshape(KC, 128, 4096).transpose(1, 0, 2)
        .astype(ml_dtypes.float8_e4m3).reshape(128, -1))

    smask = np.zeros((128, 2 * L), np.float32)
    for r in range(128):
        smask[r, (r % 2) * L:(r % 2) * L + L] = 1.0
    rmask = np.zeros((128, 512), np.float32)
    for v in range(128):
        rmask[v, (v % 4) * 128:(v % 4 + 1) * 128] = 1.0
    repl = np.zeros((128, 128), np.float32)
    for n in range(NL):
        for blk in range(KC):
            repl[32 * (n // 4) + n % 4, 8 * n + blk] = 0.5

    in_maps = []
    for k in range(NCORES):
        s0 = NL * k
        Af = A[s0:s0 + NL].reshape(NL, H, L)
        T1 = Af.reshape(NL, KC, 128, L)                      # [n, blk, hh, l]
        a1 = (2.0 * T1.transpose(2, 1, 0, 3)).astype(
            ml_dtypes.float8_e3m4).reshape(128, -1)          # [hh, c, n, l]
        Afp = np.zeros((NL, KC, 128, 256), np.float32)
        Afp[..., :L] = T1
        a2 = (2.0 * Afp.reshape(NL, KC, 128, 2, 128)
              .transpose(4, 3, 0, 1, 2)).astype(
            ml_dtypes.float8_e3m4).reshape(128, -1)          # [lp, lc, n, blk, hh]
        xs = xwb_full[s0:s0 + NL, :tsteps].transpose(1, 0, 2)  # [t, n, 4096]
        xsc = np.zeros((tsteps, 4, 32, 1024), np.float32)
        xsc[:, :, :NL, :] = xs.reshape(tsteps, NL, 4, 1024).transpose(0, 2, 1, 3)
        h0 = Af.mean(-1)                                     # [n, 1024]
        h0t = np.ascontiguousarray(
            h0.reshape(NL, KC, 128).transpose(2, 1, 0).reshape(128, 128))
        in_maps.append({
            "a1": np.ascontiguousarray(a1),
            "a2": np.ascontiguousarray(a2),
            "wh": WHs,
            "wa": WAs,
            "xwb": _bf(xsc.reshape(tsteps, 128, 1024)),
            "h0t": h0t.astype(np.float32),
            "repl": repl,
            "smask": smask,
            "rmask": rmask,
        })
    return in_maps


def kernel(x, A, Wx, Wh, Wattn, b, _tsteps=None):
    tsteps = _tsteps or TSTEPS
    nc = _get_nc(tsteps)
    in_maps = prepare_inputs(x, A, Wx, Wh, Wattn, b, tsteps)
    res = run_bass_kernel_spmd(nc, in_maps, core_ids=list(range(NCORES)))
    out = np.empty((N, tsteps, H), np.float32)
    for k in range(NCORES):
        ho = res.results[k]["hout"].astype(np.float32)       # [t, hh, (kk,n)]
        out[NL * k:NL * (k + 1)] = (
            ho.reshape(tsteps, 128, KC, NL).transpose(3, 0, 2, 1)
            .reshape(NL, tsteps, H))
    if tsteps == T:
        return out
    full = np.zeros((N, T, H), np.float32)
    full[:, :tsteps] = out
    return full


# revision 10
# speedup vs baseline: 5.9492x; 1.0570x over previous
"""Trainium2 Bass kernel for nn_CaptioningRNN (attention LSTM, T=64 steps).

Strategy: PURE DATA-PARALLEL over N (16 samples/core, ZERO collectives).
The baseline TP design paid 2 serialized AllGathers per step (~100-190us/step
of collective latency); here every core runs its 16 samples' full recurrence
independently and only the final output is gathered on the host.

Per-core residents (SBUF, per-partition budget 192KB):
  A1  [hh, (chunk c, n, l)]      E3M4 (stores 2A)   24.5KB  - scores rhs
  A2  [lp, (lc, v=8n+blk, hh)]   E3M4 (stores 2A)   32KB    - readout rhs
  Wh  [p,  (k, 4096)]            bf16               64KB    - gates rhs
  Wa  [p,  (k, 4096)]            E4M3               32KB    - gates rhs
Mixed-dtype matmuls (bf16 lhsT x fp8 rhs) are verified exact on HW, so all
lhsT operands (h^T, exp-weights, attn^T) stay bf16.

Gates: out[16, 4096] via 4x PE column tiling (tile j = gate quarter j at
psum rows 32j..32j+16), 17 chunks each (16 z-chunks + xwb-via-identity).
LSTM pointwise runs in TRANSPOSED layout [128 h, (k, n)]: the four gate
quarters are PE-transposed per 128-chunk, which sidesteps the DVE
equal-partition-base restriction and directly yields h^T for the next
step's lhsT. Softmax is max-subtracted; 1/sum is applied post-readout via
a 0/1 replication matmul that broadcasts the per-sample reciprocal to the
128 virtual (n, blk) rows.

Numerics validated by simulation: rel ~5e-3 (gate 2e-2).
"""

import os
from contextlib import ExitStack

import numpy as np
import ml_dtypes

import concourse.bass as bass
import concourse.tile as tile
from concourse import bacc, mybir
from concourse.bass_utils import run_bass_kernel_spmd
from concourse.masks import make_identity

F32 = mybir.dt.float32
BF16 = mybir.dt.bfloat16
E3 = mybir.dt.float8e3
E4 = mybir.dt.float8e4
AF = mybir.ActivationFunctionType
OP = mybir.AluOpType

N, T, D, H = 128, 64, 512, 1024
L = 196
NCORES = 8
NL = N // NCORES          # 16 samples per core
HS = 128                  # kept for test.py's empty-kernel shape
KC = 8                    # h chunks of 128
SCALE = 1.0 / float(np.sqrt(H))

TSTEPS = int(os.environ.get("KERNEL_TSTEPS", T))
ABL = os.environ.get("KERNEL_ABL", "")  # csv: noatt,nogates,nopoint,noscore,noread


def _ap(t, dims, offset=0):
    a = t[:]
    return bass.AP(a.tensor, a.offset + offset, [a.ap[0]] + dims)


def _app(tsl, dims, offset=0):
    """AP from a tile slice (keeps partition dim of the slice)."""
    return bass.AP(tsl.tensor, tsl.offset + offset, [tsl.ap[0]] + dims)


def build_nc(tsteps, repeat=1):
    nc = bacc.Bacc("TRN2", target_bir_lowering=False, debug=False,
                   num_devices=NCORES)
    d_a1 = nc.dram_tensor("a1", (128, KC * NL * L), E3, kind="ExternalInput").ap()
    d_a2 = nc.dram_tensor("a2", (128, 2 * 128 * 128), E3, kind="ExternalInput").ap()
    d_wh = nc.dram_tensor("wh", (128, KC * 4096), BF16, kind="ExternalInput").ap()
    d_wa = nc.dram_tensor("wa", (128, KC * 4096), E4, kind="ExternalInput").ap()
    d_xwb = nc.dram_tensor("xwb", (tsteps, 128, 1024), BF16,
                           kind="ExternalInput").ap()
    d_h0t = nc.dram_tensor("h0t", (128, 128), F32, kind="ExternalInput").ap()
    d_repl = nc.dram_tensor("repl", (128, 128), F32, kind="ExternalInput").ap()
    d_sm = nc.dram_tensor("smask", (128, 2 * L), F32, kind="ExternalInput").ap()
    d_rm = nc.dram_tensor("rmask", (128, 512), F32, kind="ExternalInput").ap()
    d_out = nc.dram_tensor("hout", (tsteps, 128, 128), BF16,
                           kind="ExternalOutput").ap()

    with tile.TileContext(nc) as tc:
        with ExitStack() as ctx:
            _build(ctx, tc, tsteps, d_a1, d_a2, d_wh, d_wa, d_xwb, d_h0t,
                   d_repl, d_sm, d_rm, d_out, repeat)
    nc.compile()
    return nc


def _build(ctx, tc, tsteps, d_a1, d_a2, d_wh, d_wa, d_xwb, d_h0t, d_repl,
           d_sm, d_rm, d_out, repeat=1):
    nc = tc.nc
    pp = ctx.enter_context(tc.tile_pool(name="persist", bufs=1))
    sb = ctx.enter_context(tc.tile_pool(name="work", bufs=2))
    sx = ctx.enter_context(tc.tile_pool(name="xwb", bufs=2))
    ps_g = ctx.enter_context(tc.tile_pool(name="ps_g", bufs=1, space="PSUM"))
    ps_a = ctx.enter_context(tc.tile_pool(name="ps_a", bufs=1, space="PSUM"))
    ps_s = ctx.enter_context(tc.tile_pool(name="ps_s", bufs=1, space="PSUM"))
    ps_r = ctx.enter_context(tc.tile_pool(name="ps_r", bufs=1, space="PSUM"))
    ps_w = ctx.enter_context(tc.tile_pool(name="ps_w", bufs=1, space="PSUM"))

    # ---- persistent tiles
    t_a1 = pp.tile([128, KC * NL * L], E3)
    t_a2 = pp.tile([128, 2 * 128 * 128], E3)
    t_wh = pp.tile([128, KC * 4096], BF16)
    t_wa = pp.tile([128, KC * 4096], E4)
    t_sm = pp.tile([128, 2 * L], F32)
    t_rm = pp.tile([128, 512], F32)
    t_repl = pp.tile([128, 128], F32)
    t_hbd = pp.tile([128, 2048], BF16)   # scores lhsT block-diag slabs
    t_wbd = pp.tile([128, 2048], BF16)   # readout lhsT block-diag slabs
    t_hT = pp.tile([128, 128], BF16)     # h^T compact (k, n) = gates lhsT
    t_zTa = pp.tile([128, 128], BF16)    # attn^T compact (blk, n)
    t_cT = pp.tile([128, 128], F32)      # c^T state
    t_id = pp.tile([128, 128], BF16)

    # ---- loads + one-time init
    nc.sync.dma_start(t_a1[:], d_a1)
    nc.sync.dma_start(t_a2[:], d_a2)
    nc.sync.dma_start(t_wh[:], d_wh)
    nc.sync.dma_start(t_wa[:], d_wa)
    nc.sync.dma_start(t_sm[:], d_sm)
    nc.sync.dma_start(t_rm[:], d_rm)
    nc.sync.dma_start(t_repl[:], d_repl)
    h0t = pp.tile([128, 128], F32)
    nc.sync.dma_start(h0t[:], d_h0t)
    make_identity(nc, t_id[:])
    nc.vector.memset(t_hbd[:], 0.0)
    nc.vector.memset(t_wbd[:], 0.0)

    g_ps = ps_g.tile([128, 1024], F32)
    aT_ps = ps_a.tile([128, KC * 128], BF16)
    s_ps = ps_s.tile([128, 512], F32)
    r_ps = ps_r.tile([128, 512], F32)
    wT_ps = ps_w.tile([128, 256], BF16, tag="wT")
    attnT_ps = ps_w.tile([128, 128], BF16, tag="aT")
    recv_ps = ps_w.tile([128, 8], F32, tag="rv")
    nc.vector.memset(g_ps[:], 0.0)
    nc.vector.memset(s_ps[:], 0.0)

    nc.vector.tensor_copy(t_cT[:], h0t[:])
    nc.vector.tensor_copy(t_hT[:], h0t[:])

    def fill_hbd():
        """t_hbd slab (q, c) at cols 32*(8q+c); sample n=2q'+... of group q at
        slab col 2(q%2) + (n%2). src = t_hT cols 16c + n.
        Per chunk c: iterate (q2, b, j): q = 2*q2 + b, n = 2q + j:
          dst col = 512*q2 + 258*b + 32*c + j   (258 = 8*32 + 2)
          src col = 16*c + 4*q2 + 2*b + j
        """
        for c in range(KC):
            src = _ap(t_hT, [[4, 4], [2, 2], [1, 2]], offset=16 * c)
            dst = _ap(t_hbd, [[512, 4], [258, 2], [1, 2]], offset=32 * c)
            nc.vector.tensor_copy(dst, src)

    def fill_wbd():
        """t_wbd slab for group g=2n+bh at cols lc*1024 + 32g, lanes at
        colpos 4*(g%8)+lane. dst col = lc*1024 + 256*n2 + 72*r + 36*bh + lane
        (n = 4*n2 + r). src = wT_ps col 32*n2 + r (stride-0 over bh, lane)."""
        for lc in range(2):
            rows = 128 if lc == 0 else 68
            srcsl = wT_ps[0:rows, 128 * lc:128 * (lc + 1)]
            src = _app(srcsl, [[32, 4], [1, 4], [0, 2], [0, 4]])
            dstsl = t_wbd[0:rows, 1024 * lc:1024 * (lc + 1)]
            dst = _app(dstsl, [[256, 4], [72, 4], [36, 2], [1, 4]])
            nc.vector.tensor_copy(dst, src)

    def scores_mms():
        for jc in range(4):
            for b in range(2):
                q = 2 * jc + b
                for c in range(KC):
                    nc.tensor.matmul(
                        s_ps[32 * jc:32 * jc + 32, 0:2 * L],
                        t_hbd[:, 32 * (8 * q + c):32 * (8 * q + c) + 32],
                        t_a1[:, (c * NL + 2 * q) * L:(c * NL + 2 * q + 2) * L],
                        start=(b == 0 and c == 0), stop=(b == 1 and c == KC - 1),
                        tile_position=(0, 32 * jc))

    def readout_mms():
        for a in range(4):
            for gg in range(8):
                g = 8 * a + gg
                for lc in range(2):
                    nc.tensor.matmul(
                        r_ps[32 * a:32 * a + 32, :],
                        t_wbd[:, 1024 * lc + 32 * g:1024 * lc + 32 * g + 32],
                        t_a2[:, (128 * lc + 4 * g) * 128:(128 * lc + 4 * g + 4) * 128],
                        start=(gg == 0 and lc == 0), stop=(gg == 7 and lc == 1),
                        tile_position=(0, 32 * a))

    def gates_h(xw):
        for k in range(KC):
            for j in range(4):
                for h2 in range(2):
                    nc.tensor.matmul(
                        g_ps[32 * j:32 * j + 16, 512 * h2:512 * h2 + 512],
                        t_hT[:, 16 * k:16 * k + 16],
                        t_wh[:, k * 4096 + 1024 * j + 512 * h2:
                             k * 4096 + 1024 * j + 512 * h2 + 512],
                        start=(k == 0), stop=False, tile_position=(0, 32 * j))
        for j in range(4):
            for h2 in range(2):
                nc.tensor.matmul(
                    g_ps[32 * j:32 * j + 16, 512 * h2:512 * h2 + 512],
                    t_id[32 * j:32 * j + 16, 32 * j:32 * j + 16],
                    xw[32 * j:32 * j + 16, 512 * h2:512 * h2 + 512],
                    start=False, stop=False, tile_position=(32 * j, 32 * j))

    def gates_attn():
        for k in range(KC):
            for j in range(4):
                for h2 in range(2):
                    nc.tensor.matmul(
                        g_ps[32 * j:32 * j + 16, 512 * h2:512 * h2 + 512],
                        t_zTa[:, 16 * k:16 * k + 16],
                        t_wa[:, k * 4096 + 1024 * j + 512 * h2:
                             k * 4096 + 1024 * j + 512 * h2 + 512],
                        start=False, stop=(k == KC - 1), tile_position=(0, 32 * j))

    def gates_mms(xw):
        gates_h(xw)
        gates_attn()

    def quarter(q):
        return _app(aT_ps[:], [[128, KC], [1, 16]], offset=32 * q)

    def pointwise(t, last):
        g_sb = sb.tile([128, 1024], BF16, tag="g_sb")
        nc.scalar.activation(g_sb[96:112, :], g_ps[96:112, :], AF.Tanh)
        nc.scalar.activation(g_sb[0:80, :], g_ps[0:80, :], AF.Sigmoid)
        for k in range(KC):
            nc.tensor.transpose(aT_ps[:, 128 * k:128 * (k + 1)],
                                g_sb[:, 128 * k:128 * (k + 1)], t_id[:])
        gTs = sb.tile([128, 128], BF16, tag="gTs")
        nc.vector.tensor_copy(gTs[:], quarter(3))
        c1 = sb.tile([128, 128], F32, tag="c1")
        nc.vector.tensor_tensor(c1[:], quarter(1), t_cT[:], op=OP.mult)
        c2 = sb.tile([128, 128], F32, tag="c2")
        nc.vector.tensor_tensor(c2[:], quarter(0), gTs[:], op=OP.mult)
        nc.vector.tensor_add(t_cT[:], c1[:], c2[:])
        tch = sb.tile([128, 128], F32, tag="tch")
        nc.scalar.activation(tch[:], t_cT[:], AF.Tanh)
        nc.vector.tensor_tensor(t_hT[:], quarter(2), tch[:], op=OP.mult)
        nc.scalar.dma_start(d_out[t], t_hT[:])
        if not last:
            fill_hbd()

    fill_hbd()

    def load_xw(t):
        xw = sx.tile([128, 1024], BF16, tag="xw")
        nc.sync.dma_start(xw[:], d_xwb[t])
        return xw

    xw_next = load_xw(0)

    for rep in range(repeat):
     for t in range(tsteps):
        last = (t == tsteps - 1 and rep == repeat - 1)
        # ---- xwb prefetch: consume this step's buffer, start next step's DMA
        xw = xw_next
        if not last:
            xw_next = load_xw((t + 1) % tsteps)

        if "noatt" in ABL:
            gates_mms(xw)
            pointwise(t, last)
            continue
        # ---- scores (uses t_hbd from previous step's h)
        if "noscore" not in ABL:
            scores_mms()
        # ---- gates h-part: PE busy while DVE/ACT run extract+softmax
        if "nogates" not in ABL:
            gates_h(xw)
        stmp = sb.tile([128, 2 * L], F32, tag="stmp")
        nc.vector.tensor_tensor(stmp[:], s_ps[:, 0:2 * L], t_sm[:], op=OP.mult)
        sc = sb.tile([128, L], F32, tag="sc")
        nc.vector.tensor_reduce(sc[:], _ap(stmp, [[1, L], [L, 2]]),
                                axis=mybir.AxisListType.X, op=OP.add)
        # ---- softmax (max-subtracted, unnormalized; psum holds 2*s)
        m = sb.tile([128, 1], F32, tag="m")
        nc.vector.tensor_reduce(m[:], sc[:], axis=mybir.AxisListType.X, op=OP.max)
        nb = sb.tile([128, 1], F32, tag="nb")
        nc.vector.tensor_scalar_mul(nb[:], m[:], -SCALE / 2.0)
        wexp = sb.tile([128, L], BF16, tag="wexp")
        esum = sb.tile([128, 1], F32, tag="esum")
        nc.scalar.activation(wexp[:], sc[:], AF.Exp, bias=nb[:], scale=SCALE / 2.0,
                             accum_out=esum[:])
        rec = sb.tile([128, 1], F32, tag="rec")
        nc.vector.reciprocal(rec[:], esum[:])
        # ---- w^T transposes + rec_v replication (PE, tiny)
        nc.tensor.transpose(wT_ps[:, 0:128], wexp[:, 0:128], t_id[:])
        nc.tensor.transpose(wT_ps[0:68, 128:256], wexp[:, 128:L], t_id[:])
        nc.tensor.matmul(recv_ps[:, 0:1], t_repl[:], rec[:], start=True,
                         stop=True)
        recv = sb.tile([128, 1], F32, tag="recv")
        nc.vector.tensor_copy(recv[:], recv_ps[:, 0:1])
        fill_wbd()
        # ---- readout -> attn
        if "noread" not in ABL:
            readout_mms()
        rtmp = sb.tile([128, 512], F32, tag="rtmp")
        nc.vector.tensor_tensor(rtmp[:], r_ps[:], t_rm[:], op=OP.mult)
        attnf = sb.tile([128, 128], F32, tag="attnf")
        nc.vector.tensor_reduce(attnf[:], _ap(rtmp, [[1, 128], [128, 4]]),
                                axis=mybir.AxisListType.X, op=OP.add)
        attnb = sb.tile([128, 128], BF16, tag="attnb")
        nc.vector.tensor_scalar_mul(attnb[:], attnf[:], recv[:])
        nc.tensor.transpose(attnT_ps[:], attnb[:], t_id[:])
        # zTa compact (blk, n): dst col 16*blk + n <- src col 8*n + blk
        nc.vector.tensor_copy(_ap(t_zTa, [[16, 8], [1, 16]]),
                              _app(attnT_ps[:], [[1, 8], [8, 16]]))
        # ---- gates attn-part
        if "nogates" not in ABL:
            gates_attn()
        pointwise(t, last)

# ---------------------------------------------------------------------------
# host side
# ---------------------------------------------------------------------------
_NC_CACHE = {}


def _get_nc(tsteps, repeat=1):
    key = (tsteps, repeat)
    if key not in _NC_CACHE:
        _NC_CACHE[key] = build_nc(tsteps, repeat)
    return _NC_CACHE[key]


def _bf(v):
    return v.astype(ml_dtypes.bfloat16)


def prepare_inputs(x, A, Wx, Wh, Wattn, b, tsteps):
    x = np.asarray(x, np.float32)
    A = np.asarray(A, np.float32)
    Wh = np.asarray(Wh, np.float32)
    Wattn = np.asarray(Wattn, np.float32)
    xwb_full = (_bf(x.reshape(N * T, D)).astype(np.float32)
                @ _bf(np.asarray(Wx, np.float32)).astype(np.float32)
                ).reshape(N, T, 4 * H) + np.asarray(b, np.float32)[None, None, :]

    WHs = np.ascontiguousarray(
        _bf(Wh.reshape(KC, 128, 4096).transpose(1, 0, 2)).reshape(128, -1))
    WAs = np.ascontiguousarray(
        Wattn.reshape(KC, 128, 4096).transpose(1, 0, 2)
        .astype(ml_dtypes.float8_e4m3).reshape(128, -1))

    smask = np.zeros((128, 2 * L), np.float32)
    for r in range(128):
        smask[r, (r % 2) * L:(r % 2) * L + L] = 1.0
    rmask = np.zeros((128, 512), np.float32)
    for v in range(128):
        rmask[v, (v % 4) * 128:(v % 4 + 1) * 128] = 1.0
    repl = np.zeros((128, 128), np.float32)
    for n in range(NL):
        for blk in range(KC):
            repl[32 * (n // 4) + n % 4, 8 * n + blk] = 0.5

    in_maps = []
    for k in range(NCORES):
        s0 = NL * k
        Af = A[s0:s0 + NL].reshape(NL, H, L)
        T1 = Af.reshape(NL, KC, 128, L)                      # [n, blk, hh, l]
        a1 = (2.0 * T1.transpose(2, 1, 0, 3)).astype(
            ml_dtypes.float8_e3m4).reshape(128, -1)          # [hh, c, n, l]
        Afp = np.zeros((NL, KC, 128, 256), np.float32)
        Afp[..., :L] = T1
        a2 = (2.0 * Afp.reshape(NL, KC, 128, 2, 128)
              .transpose(4, 3, 0, 1, 2)).astype(
            ml_dtypes.float8_e3m4).reshape(128, -1)          # [lp, lc, n, blk, hh]
        xs = xwb_full[s0:s0 + NL, :tsteps].transpose(1, 0, 2)  # [t, n, 4096]
        xsc = np.zeros((tsteps, 4, 32, 1024), np.float32)
        xsc[:, :, :NL, :] = xs.reshape(tsteps, NL, 4, 1024).transpose(0, 2, 1, 3)
        h0 = Af.mean(-1)                                     # [n, 1024]
        h0t = np.ascontiguousarray(
            h0.reshape(NL, KC, 128).transpose(2, 1, 0).reshape(128, 128))
        in_maps.append({
            "a1": np.ascontiguousarray(a1),
            "a2": np.ascontiguousarray(a2),
            "wh": WHs,
            "wa": WAs,
            "xwb": _bf(xsc.reshape(tsteps, 128, 1024)),
            "h0t": h0t.astype(np.float32),
            "repl": repl,
            "smask": smask,
            "rmask": rmask,
        })
    return in_maps


def kernel(x, A, Wx, Wh, Wattn, b, _tsteps=None):
    tsteps = _tsteps or TSTEPS
    nc = _get_nc(tsteps)
    in_maps = prepare_inputs(x, A, Wx, Wh, Wattn, b, tsteps)
    res = run_bass_kernel_spmd(nc, in_maps, core_ids=list(range(NCORES)))
    out = np.empty((N, tsteps, H), np.float32)
    for k in range(NCORES):
        ho = res.results[k]["hout"].astype(np.float32)       # [t, hh, (kk,n)]
        out[NL * k:NL * (k + 1)] = (
            ho.reshape(tsteps, 128, KC, NL).transpose(3, 0, 2, 1)
            .reshape(NL, tsteps, H))
    if tsteps == T:
        return out
    full = np.zeros((N, T, H), np.float32)
    full[:, :tsteps] = out
    return full


# revision 11
# speedup vs baseline: 12.4986x; 2.1009x over previous
"""Trainium2 Bass kernel for nn_CaptioningRNN (attention LSTM, T=64 steps).

Strategy: PURE DATA-PARALLEL over N (16 samples/core, ZERO collectives).
The baseline TP design paid 2 serialized AllGathers per step (~100-190us/step
of collective latency); here every core runs its 16 samples' full recurrence
independently and only the final output is gathered on the host.

Per-core residents (SBUF, per-partition budget 192KB):
  A1  [hh, (chunk c, n, l)]      E3M4 (stores 2A)   24.5KB  - scores rhs
  A2  [lp, (lc, v=8n+blk, hh)]   E3M4 (stores 2A)   32KB    - readout rhs
  Wh  [p,  (k, 4096)]            bf16               64KB    - gates rhs
  Wa  [p,  (k, 4096)]            E4M3               32KB    - gates rhs
Mixed-dtype matmuls (bf16 lhsT x fp8 rhs) are verified exact on HW, so all
lhsT operands (h^T, exp-weights, attn^T) stay bf16.

Gates: out[16, 4096] via 4x PE column tiling (tile j = gate quarter j at
psum rows 32j..32j+16), 17 chunks each (16 z-chunks + xwb-via-identity).
LSTM pointwise runs in TRANSPOSED layout [128 h, (k, n)]: the four gate
quarters are PE-transposed per 128-chunk, which sidesteps the DVE
equal-partition-base restriction and directly yields h^T for the next
step's lhsT. Softmax is max-subtracted; 1/sum is applied post-readout via
a 0/1 replication matmul that broadcasts the per-sample reciprocal to the
128 virtual (n, blk) rows.

Numerics validated by simulation: rel ~5e-3 (gate 2e-2).
"""

import os
from contextlib import ExitStack

import numpy as np
import ml_dtypes

import concourse.bass as bass
import concourse.tile as tile
from concourse import bacc, mybir
from concourse.bass_utils import run_bass_kernel_spmd
from concourse.masks import make_identity

F32 = mybir.dt.float32
BF16 = mybir.dt.bfloat16
E3 = mybir.dt.float8e3
E4 = mybir.dt.float8e4
AF = mybir.ActivationFunctionType
OP = mybir.AluOpType

N, T, D, H = 128, 64, 512, 1024
L = 196
NCORES = 8
NL = N // NCORES          # 16 samples per core
HS = 128                  # kept for test.py's empty-kernel shape
KC = 8                    # h chunks of 128
SCALE = 1.0 / float(np.sqrt(H))

TSTEPS = int(os.environ.get("KERNEL_TSTEPS", T))
ABL = os.environ.get("KERNEL_ABL", "")  # csv: noatt,nogates,nopoint,noscore,noread


def _ap(t, dims, offset=0):
    a = t[:]
    return bass.AP(a.tensor, a.offset + offset, [a.ap[0]] + dims)


def _app(tsl, dims, offset=0):
    """AP from a tile slice (keeps partition dim of the slice)."""
    return bass.AP(tsl.tensor, tsl.offset + offset, [tsl.ap[0]] + dims)


def build_nc(tsteps, repeat=1):
    nc = bacc.Bacc("TRN2", target_bir_lowering=False, debug=False,
                   num_devices=NCORES)
    d_a1 = nc.dram_tensor("a1", (128, KC * NL * L), E3, kind="ExternalInput").ap()
    d_a2 = nc.dram_tensor("a2", (128, 2 * 128 * 128), E3, kind="ExternalInput").ap()
    d_wh = nc.dram_tensor("wh", (128, KC * 4096), BF16, kind="ExternalInput").ap()
    d_wa = nc.dram_tensor("wa", (128, KC * 4096), E4, kind="ExternalInput").ap()
    d_xwb = nc.dram_tensor("xwb", (tsteps, 128, 1024), BF16,
                           kind="ExternalInput").ap()
    d_h0t = nc.dram_tensor("h0t", (128, 128), F32, kind="ExternalInput").ap()
    d_repl = nc.dram_tensor("repl", (128, 128), F32, kind="ExternalInput").ap()
    d_sm = nc.dram_tensor("smask", (128, 2 * L), F32, kind="ExternalInput").ap()
    d_rm = nc.dram_tensor("rmask", (128, 512), F32, kind="ExternalInput").ap()
    d_out = nc.dram_tensor("hout", (tsteps, 128, 128), BF16,
                           kind="ExternalOutput").ap()

    with tile.TileContext(nc) as tc:
        with ExitStack() as ctx:
            _build(ctx, tc, tsteps, d_a1, d_a2, d_wh, d_wa, d_xwb, d_h0t,
                   d_repl, d_sm, d_rm, d_out, repeat)
    nc.compile()
    return nc


def _build(ctx, tc, tsteps, d_a1, d_a2, d_wh, d_wa, d_xwb, d_h0t, d_repl,
           d_sm, d_rm, d_out, repeat=1):
    nc = tc.nc
    pp = ctx.enter_context(tc.tile_pool(name="persist", bufs=1))
    sb = ctx.enter_context(tc.tile_pool(name="work", bufs=2))
    sx = ctx.enter_context(tc.tile_pool(name="xwb", bufs=2))
    ps_g = ctx.enter_context(tc.tile_pool(name="ps_g", bufs=1, space="PSUM"))
    ps_a = ctx.enter_context(tc.tile_pool(name="ps_a", bufs=1, space="PSUM"))
    ps_s = ctx.enter_context(tc.tile_pool(name="ps_s", bufs=1, space="PSUM"))
    ps_r = ctx.enter_context(tc.tile_pool(name="ps_r", bufs=1, space="PSUM"))
    ps_w = ctx.enter_context(tc.tile_pool(name="ps_w", bufs=1, space="PSUM"))

    # ---- persistent tiles
    t_a1 = pp.tile([128, KC * NL * L], E3)
    t_a2 = pp.tile([128, 2 * 128 * 128], E3)
    t_wh = pp.tile([128, KC * 4096], BF16)
    t_wa = pp.tile([128, KC * 4096], E4)
    t_sm = pp.tile([128, 2 * L], F32)
    t_rm = pp.tile([128, 512], F32)
    t_repl = pp.tile([128, 128], F32)
    t_hbd = pp.tile([128, 2048], BF16)   # scores lhsT block-diag slabs
    t_wbd = pp.tile([128, 2048], BF16)   # readout lhsT block-diag slabs
    t_hT = pp.tile([128, 128], BF16)     # h^T compact (k, n) = gates lhsT
    t_zTa = pp.tile([128, 128], BF16)    # attn^T compact (blk, n)
    t_cT = pp.tile([128, 128], F32)      # c^T state
    t_id = pp.tile([128, 128], BF16)

    # ---- loads + one-time init
    nc.sync.dma_start(t_a1[:], d_a1)
    nc.sync.dma_start(t_a2[:], d_a2)
    nc.sync.dma_start(t_wh[:], d_wh)
    nc.sync.dma_start(t_wa[:], d_wa)
    nc.sync.dma_start(t_sm[:], d_sm)
    nc.sync.dma_start(t_rm[:], d_rm)
    nc.sync.dma_start(t_repl[:], d_repl)
    h0t = pp.tile([128, 128], F32)
    nc.sync.dma_start(h0t[:], d_h0t)
    make_identity(nc, t_id[:])
    nc.vector.memset(t_hbd[:], 0.0)
    nc.vector.memset(t_wbd[:], 0.0)

    g_ps = ps_g.tile([128, 1024], F32)
    aT_ps = ps_a.tile([128, KC * 128], BF16)
    s_ps = ps_s.tile([128, 512], F32)
    r_ps = ps_r.tile([128, 512], F32)
    wT_ps = ps_w.tile([128, 256], BF16, tag="wT")
    attnT_ps = ps_w.tile([128, 128], BF16, tag="aT")
    recv_ps = ps_w.tile([128, 8], F32, tag="rv")
    nc.vector.memset(g_ps[:], 0.0)
    nc.vector.memset(s_ps[:], 0.0)

    nc.vector.tensor_copy(t_cT[:], h0t[:])
    nc.vector.tensor_copy(t_hT[:], h0t[:])

    def fill_hbd():
        """t_hbd slab (q, c) at cols 32*(8q+c); sample n=2q'+... of group q at
        slab col 2(q%2) + (n%2). src = t_hT cols 16c + n.
        Per chunk c: iterate (q2, b, j): q = 2*q2 + b, n = 2q + j:
          dst col = 512*q2 + 258*b + 32*c + j   (258 = 8*32 + 2)
          src col = 16*c + 4*q2 + 2*b + j
        """
        for c in range(KC):
            src = _ap(t_hT, [[4, 4], [2, 2], [1, 2]], offset=16 * c)
            dst = _ap(t_hbd, [[512, 4], [258, 2], [1, 2]], offset=32 * c)
            nc.vector.tensor_copy(dst, src)

    def fill_wbd():
        """t_wbd slab for group g=2n+bh at cols lc*1024 + 32g, lanes at
        colpos 4*(g%8)+lane. dst col = lc*1024 + 256*n2 + 72*r + 36*bh + lane
        (n = 4*n2 + r). src = wT_ps col 32*n2 + r (stride-0 over bh, lane)."""
        for lc in range(2):
            rows = 128 if lc == 0 else 68
            srcsl = wT_ps[0:rows, 128 * lc:128 * (lc + 1)]
            src = _app(srcsl, [[32, 4], [1, 4], [0, 2], [0, 4]])
            dstsl = t_wbd[0:rows, 1024 * lc:1024 * (lc + 1)]
            dst = _app(dstsl, [[256, 4], [72, 4], [36, 2], [1, 4]])
            nc.vector.tensor_copy(dst, src)

    def scores_mms():
        # quadrant (jc) innermost so consecutive MMs hit different col-tiles
        # and stream concurrently (PE starts MMs in order; same-tile MMs block)
        for b in range(2):
            for c in range(KC):
                for jc in range(4):
                    q = 2 * jc + b
                    nc.tensor.matmul(
                        s_ps[32 * jc:32 * jc + 32, 0:2 * L],
                        t_hbd[:, 32 * (8 * q + c):32 * (8 * q + c) + 32],
                        t_a1[:, (c * NL + 2 * q) * L:(c * NL + 2 * q + 2) * L],
                        start=(b == 0 and c == 0), stop=(b == 1 and c == KC - 1),
                        tile_position=(0, 32 * jc))

    def readout_mms():
        for lc in range(2):
            for gg in range(8):
                for a in range(4):
                    g = 8 * a + gg
                    nc.tensor.matmul(
                        r_ps[32 * a:32 * a + 32, :],
                        t_wbd[:, 1024 * lc + 32 * g:1024 * lc + 32 * g + 32],
                        t_a2[:, (128 * lc + 4 * g) * 128:(128 * lc + 4 * g + 4) * 128],
                        start=(lc == 0 and gg == 0), stop=(lc == 1 and gg == 7),
                        tile_position=(0, 32 * a))

    def gates_h(xw):
        for h2 in range(2):
            for k in range(KC):
                for j in range(4):
                    nc.tensor.matmul(
                        g_ps[32 * j:32 * j + 16, 512 * h2:512 * h2 + 512],
                        t_hT[:, 16 * k:16 * k + 16],
                        t_wh[:, k * 4096 + 1024 * j + 512 * h2:
                             k * 4096 + 1024 * j + 512 * h2 + 512],
                        start=(k == 0), stop=False, tile_position=(0, 32 * j))
        for j in range(4):
            for h2 in range(2):
                nc.tensor.matmul(
                    g_ps[32 * j:32 * j + 16, 512 * h2:512 * h2 + 512],
                    t_id[32 * j:32 * j + 16, 32 * j:32 * j + 16],
                    xw[32 * j:32 * j + 16, 512 * h2:512 * h2 + 512],
                    start=False, stop=False, tile_position=(32 * j, 32 * j))

    def gates_attn():
        for h2 in range(2):
            for k in range(KC):
                for j in range(4):
                    nc.tensor.matmul(
                        g_ps[32 * j:32 * j + 16, 512 * h2:512 * h2 + 512],
                        t_zTa[:, 16 * k:16 * k + 16],
                        t_wa[:, k * 4096 + 1024 * j + 512 * h2:
                             k * 4096 + 1024 * j + 512 * h2 + 512],
                        start=False, stop=(k == KC - 1), tile_position=(0, 32 * j))

    def gates_mms(xw):
        gates_h(xw)
        gates_attn()

    def quarter(q):
        return _app(aT_ps[:], [[128, KC], [1, 16]], offset=32 * q)

    def pointwise(t, last):
        g_sb = sb.tile([128, 1024], BF16, tag="g_sb")
        nc.scalar.activation(g_sb[96:112, :], g_ps[96:112, :], AF.Tanh)
        nc.scalar.activation(g_sb[0:80, :], g_ps[0:80, :], AF.Sigmoid)
        for k in range(KC):
            nc.tensor.transpose(aT_ps[:, 128 * k:128 * (k + 1)],
                                g_sb[:, 128 * k:128 * (k + 1)], t_id[:])
        gTs = sb.tile([128, 128], BF16, tag="gTs")
        nc.vector.tensor_copy(gTs[:], quarter(3))
        c1 = sb.tile([128, 128], F32, tag="c1")
        nc.vector.tensor_tensor(c1[:], quarter(1), t_cT[:], op=OP.mult)
        c2 = sb.tile([128, 128], F32, tag="c2")
        nc.vector.tensor_tensor(c2[:], quarter(0), gTs[:], op=OP.mult)
        nc.vector.tensor_add(t_cT[:], c1[:], c2[:])
        tch = sb.tile([128, 128], F32, tag="tch")
        nc.scalar.activation(tch[:], t_cT[:], AF.Tanh)
        nc.vector.tensor_tensor(t_hT[:], quarter(2), tch[:], op=OP.mult)
        nc.scalar.dma_start(d_out[t], t_hT[:])
        if not last:
            fill_hbd()

    fill_hbd()

    def load_xw(t):
        xw = sx.tile([128, 1024], BF16, tag="xw")
        nc.sync.dma_start(xw[:], d_xwb[t])
        return xw

    xw_next = load_xw(0)

    for rep in range(repeat):
     for t in range(tsteps):
        last = (t == tsteps - 1 and rep == repeat - 1)
        # ---- xwb prefetch: consume this step's buffer, start next step's DMA
        xw = xw_next
        if not last:
            xw_next = load_xw((t + 1) % tsteps)

        if "noatt" in ABL:
            gates_mms(xw)
            pointwise(t, last)
            continue
        # ---- scores (uses t_hbd from previous step's h)
        if "noscore" not in ABL:
            scores_mms()
        # ---- gates h-part: PE busy while DVE/ACT run extract+softmax
        if "nogates" not in ABL:
            gates_h(xw)
        stmp = sb.tile([128, 2 * L], F32, tag="stmp")
        nc.vector.tensor_tensor(stmp[:], s_ps[:, 0:2 * L], t_sm[:], op=OP.mult)
        sc = sb.tile([128, L], F32, tag="sc")
        nc.vector.tensor_reduce(sc[:], _ap(stmp, [[1, L], [L, 2]]),
                                axis=mybir.AxisListType.X, op=OP.add)
        # ---- softmax (max-subtracted, unnormalized; psum holds 2*s)
        m = sb.tile([128, 1], F32, tag="m")
        nc.vector.tensor_reduce(m[:], sc[:], axis=mybir.AxisListType.X, op=OP.max)
        nb = sb.tile([128, 1], F32, tag="nb")
        nc.vector.tensor_scalar_mul(nb[:], m[:], -SCALE / 2.0)
        wexp = sb.tile([128, L], BF16, tag="wexp")
        esum = sb.tile([128, 1], F32, tag="esum")
        nc.scalar.activation(wexp[:], sc[:], AF.Exp, bias=nb[:], scale=SCALE / 2.0,
                             accum_out=esum[:])
        rec = sb.tile([128, 1], F32, tag="rec")
        nc.vector.reciprocal(rec[:], esum[:])
        # ---- w^T transposes + rec_v replication (PE, tiny)
        nc.tensor.transpose(wT_ps[:, 0:128], wexp[:, 0:128], t_id[:])
        nc.tensor.transpose(wT_ps[0:68, 128:256], wexp[:, 128:L], t_id[:])
        nc.tensor.matmul(recv_ps[:, 0:1], t_repl[:], rec[:], start=True,
                         stop=True)
        recv = sb.tile([128, 1], F32, tag="recv")
        nc.vector.tensor_copy(recv[:], recv_ps[:, 0:1])
        fill_wbd()
        # ---- readout -> attn
        if "noread" not in ABL:
            readout_mms()
        rtmp = sb.tile([128, 512], F32, tag="rtmp")
        nc.vector.tensor_tensor(rtmp[:], r_ps[:], t_rm[:], op=OP.mult)
        attnf = sb.tile([128, 128], F32, tag="attnf")
        nc.vector.tensor_reduce(attnf[:], _ap(rtmp, [[1, 128], [128, 4]]),
                                axis=mybir.AxisListType.X, op=OP.add)
        attnb = sb.tile([128, 128], BF16, tag="attnb")
        nc.vector.tensor_scalar_mul(attnb[:], attnf[:], recv[:])
        nc.tensor.transpose(attnT_ps[:], attnb[:], t_id[:])
        # zTa compact (blk, n): dst col 16*blk + n <- src col 8*n + blk
        nc.vector.tensor_copy(_ap(t_zTa, [[16, 8], [1, 16]]),
                              _app(attnT_ps[:], [[1, 8], [8, 16]]))
        # ---- gates attn-part
        if "nogates" not in ABL:
            gates_attn()
        pointwise(t, last)

# ---------------------------------------------------------------------------
# host side
# ---------------------------------------------------------------------------
_NC_CACHE = {}


def _get_nc(tsteps, repeat=1):
    key = (tsteps, repeat)
    if key not in _NC_CACHE:
        _NC_CACHE[key] = build_nc(tsteps, repeat)
    return _NC_CACHE[key]


def _bf(v):
    return v.astype(ml_dtypes.bfloat16)


def prepare_inputs(x, A, Wx, Wh, Wattn, b, tsteps):
    x = np.asarray(x, np.float32)
    A = np.asarray(A, np.float32)
    Wh = np.asarray(Wh, np.float32)
    Wattn = np.asarray(Wattn, np.float32)
    xwb_full = (_bf(x.reshape(N * T, D)).astype(np.float32)
                @ _bf(np.asarray(Wx, np.float32)).astype(np.float32)
                ).reshape(N, T, 4 * H) + np.asarray(b, np.float32)[None, None, :]

    WHs = np.ascontiguousarray(
        _bf(Wh.reshape(KC, 128, 4096).transpose(1, 0, 2)).reshape(128, -1))
    WAs = np.ascontiguousarray(
        Wattn.reshape(KC, 128, 4096).transpose(1, 0, 2)
        .astype(ml_dtypes.float8_e4m3).reshape(128, -1))

    smask = np.zeros((128, 2 * L), np.float32)
    for r in range(128):
        smask[r, (r % 2) * L:(r % 2) * L + L] = 1.0
    rmask = np.zeros((128, 512), np.float32)
    for v in range(128):
        rmask[v, (v % 4) * 128:(v % 4 + 1) * 128] = 1.0
    repl = np.zeros((128, 128), np.float32)
    for n in range(NL):
        for blk in range(KC):
            repl[32 * (n // 4) + n % 4, 8 * n + blk] = 0.5

    in_maps = []
    for k in range(NCORES):
        s0 = NL * k
        Af = A[s0:s0 + NL].reshape(NL, H, L)
        T1 = Af.reshape(NL, KC, 128, L)                      # [n, blk, hh, l]
        a1 = (2.0 * T1.transpose(2, 1, 0, 3)).astype(
            ml_dtypes.float8_e3m4).reshape(128, -1)          # [hh, c, n, l]
        Afp = np.zeros((NL, KC, 128, 256), np.float32)
        Afp[..., :L] = T1
        a2 = (2.0 * Afp.reshape(NL, KC, 128, 2, 128)
              .transpose(4, 3, 0, 1, 2)).astype(
            ml_dtypes.float8_e3m4).reshape(128, -1)          # [lp, lc, n, blk, hh]
        xs = xwb_full[s0:s0 + NL, :tsteps].transpose(1, 0, 2)  # [t, n, 4096]
        xsc = np.zeros((tsteps, 4, 32, 1024), np.float32)
        xsc[:, :, :NL, :] = xs.reshape(tsteps, NL, 4, 1024).transpose(0, 2, 1, 3)
        h0 = Af.mean(-1)                                     # [n, 1024]
        h0t = np.ascontiguousarray(
            h0.reshape(NL, KC, 128).transpose(2, 1, 0).reshape(128, 128))
        in_maps.append({
            "a1": np.ascontiguousarray(a1),
            "a2": np.ascontiguousarray(a2),
            "wh": WHs,
            "wa": WAs,
            "xwb": _bf(xsc.reshape(tsteps, 128, 1024)),
            "h0t": h0t.astype(np.float32),
            "repl": repl,
            "smask": smask,
            "rmask": rmask,
        })
    return in_maps


def kernel(x, A, Wx, Wh, Wattn, b, _tsteps=None):
    tsteps = _tsteps or TSTEPS
    nc = _get_nc(tsteps)
    in_maps = prepare_inputs(x, A, Wx, Wh, Wattn, b, tsteps)
    res = run_bass_kernel_spmd(nc, in_maps, core_ids=list(range(NCORES)))
    out = np.empty((N, tsteps, H), np.float32)
    for k in range(NCORES):
        ho = res.results[k]["hout"].astype(np.float32)       # [t, hh, (kk,n)]
        out[NL * k:NL * (k + 1)] = (
            ho.reshape(tsteps, 128, KC, NL).transpose(3, 0, 2, 1)
            .reshape(NL, tsteps, H))
    if tsteps == T:
        return out
    full = np.zeros((N, T, H), np.float32)
    full[:, :tsteps] = out
    return full
